# revision 14
# baseline (speedup 1.0000x reference)
"""Ensemble-MLP (grouped 1x1 conv) Trainium2 kernel.

Computation (per batch row b):
  h = relu(x @ W0[e] + b0[e])             e = 0..9 ensembles, 256 units
  h = relu(h @ Wh[l,e] + bh[l,e])         l = 0..6 hidden layers
  y[e] = h @ Wf[e] + bf[e]                201 outputs per ensemble
  out[b, o'] = mean_j yflat[b, o'*10 + j] (strided channel mix, yflat = e*201+o)

Strategy:
  * Data parallel: batch 16384 -> 2048 rows per core on 8 cores. Weights
    replicated.
  * Activations live in SBUF transposed: H[channel, batch], channel on
    partitions (256 = 2 chunks of 128), batch on the free axis (2048).
  * Every layer is matmul(out[o, b] += W[c, o].T @ H[c, b]) with fp32r
    (full-rate fp32 PE mode, N=512 columns per PSUM bank).
  * Layer-0 bias is folded into the matmul via an all-ones row appended to
    x^T (K=7). Hidden biases ride the relu post-op (per-partition bias).
  * The final channel-mixing mean is folded into the last-layer weights on
    the host: out = sum_e H_e @ V[e] + bp, V[e] = Wf[e] @ Me[e] (exact
    linear algebra, no approximation).
  * relu post-ops alternate between ScalarE (ACT) and VectorE (DVE) so the
    PE stays the bottleneck; final-layer partials accumulate into SBUF.
  * Warmup matmuls on a zeroed tile ramp the PE p-state (0.65 -> 2.4 GHz)
    while the first weight DMAs land, so real matmuls start at full clock.
  * Ensemble-accumulation adds run on the otherwise-idle GpSimd engine
    (e0..e8), keeping ACT/DVE dedicated to relus: no post-op backlog at
    ensemble boundaries means no PE idle gap and no p-state reset.
  * Last ensemble accumulates on DVE/GpSimd and each output slice DMAs
    out as soon as it is final, shrinking the tail drain.
"""

import numpy as np
from contextlib import ExitStack

import concourse.bass as bass
import concourse.mybir as mybir
import concourse.tile as tile
from concourse import bacc, bass_utils

F32 = mybir.dt.float32
F32R = mybir.dt.float32r

ENS, N_UNITS, N_HID, IN_DIM, OUT_DIM, BATCH = 10, 256, 7, 6, 201, 16384
N_CORES = 8
BC = BATCH // N_CORES          # 2048 batch rows per core
NT = BC // 512                 # 4 moving-operand tiles of 512
OC1 = OUT_DIM - 128            # 73 rows in the second output chunk
WARMUP_MM = 14                 # PE p-state warmup matmuls during DMA head

_CACHE = {}


def build_program():
    nc = bacc.Bacc("TRN2", debug=False)

    xt = nc.dram_tensor("xt", (IN_DIM + 1, BC), F32R, kind="ExternalInput").ap()
    w0 = nc.dram_tensor("w0", (ENS, IN_DIM + 1, N_UNITS), F32R, kind="ExternalInput").ap()
    wh = nc.dram_tensor("wh", (ENS, 128, N_HID * 2 * N_UNITS), F32R, kind="ExternalInput").ap()
    bh = nc.dram_tensor("bh", (ENS, 128, N_HID * 2), F32, kind="ExternalInput").ap()
    vw = nc.dram_tensor("vw", (ENS, 128, 2 * 256), F32R, kind="ExternalInput").ap()
    bp = nc.dram_tensor("bp", (128, 2), F32, kind="ExternalInput").ap()
    yt = nc.dram_tensor("yt", (256, BC), F32, kind="ExternalOutput").ap()

    add = mybir.AluOpType.add
    mx = mybir.AluOpType.max
    relu = mybir.ActivationFunctionType.Relu

    with ExitStack() as ctx:
        tc = ctx.enter_context(tile.TileContext(nc))
        const = ctx.enter_context(tc.tile_pool(name="const", bufs=1))
        wpool = ctx.enter_context(tc.tile_pool(name="w", bufs=2))
        hpool = ctx.enter_context(tc.tile_pool(name="h", bufs=2))
        opool = ctx.enter_context(tc.tile_pool(name="acc", bufs=1))
        pspool = ctx.enter_context(tc.tile_pool(name="ps", bufs=8, space="PSUM"))

        x_t = const.tile([IN_DIM + 1, BC], F32R)
        bp_t = const.tile([128, 2], F32)
        warm_t = const.tile([128, 512], F32, tag="warm", name="warm")
        out_t = [opool.tile([128, BC], F32, tag=f"out{i}", name=f"out{i}")
                 for i in range(2)]

        # PE p-state ramps with *continuous* busy time (0.65 -> 1.2 -> 2.4
        # GHz over ~3us); any idle gap resets it. Dummy matmuls on a zeroed
        # tile keep the PE busy through the initial weight-DMA window so the
        # first real matmul already runs at full clock.
        nc.vector.memset(warm_t, 0.0)
        warm_r = warm_t.bitcast(F32R)
        for _ in range(WARMUP_MM):
            ps = pspool.tile([128, 512], F32, tag="ps")
            nc.tensor.matmul(ps, lhsT=warm_r[:, 0:128], rhs=warm_r,
                             start=True, stop=True)

        def relu_post(dst, ps, bias_ap, use_act):
            # Phase-aware engine choice. The killer stall was: ensemble e's
            # L0 relus queued on DVE *behind* e-1's final-layer adds (in-order
            # engine queue), idling the PE and resetting its p-state. So: L0
            # relus all on ACT, H0 relus all on DVE (while ACT drains the L0
            # backlog), H1..H6 alternate, final adds all on DVE.
            if use_act:
                nc.scalar.activation(out=dst, in_=ps, func=relu,
                                     bias=bias_ap if bias_ap is not None else 0.0)
            elif bias_ap is not None:
                nc.vector.tensor_scalar(out=dst, in0=ps, scalar1=bias_ap,
                                        scalar2=0.0, op0=add, op1=mx)
            else:
                nc.vector.tensor_scalar(out=dst, in0=ps, scalar1=0.0,
                                        scalar2=None, op0=mx)

        for e in range(ENS):
            w0_t = wpool.tile([IN_DIM + 1, N_UNITS], F32R, tag="w0")
            nc.sync.dma_start(out=w0_t, in_=w0[e])
            if e == 0:
                # x lands on a single DMA engine (only 7 partition rows), so
                # issue it right after w0 in b-tile slices: layer-0's first
                # matmuls only wait on their own slice.
                for bt in range(NT):
                    nc.sync.dma_start(out=x_t[:, bt * 512:(bt + 1) * 512],
                                      in_=xt[:, bt * 512:(bt + 1) * 512])
                nc.sync.dma_start(out=bp_t, in_=bp)
            wh_t = wpool.tile([128, N_HID * 2 * N_UNITS], F32R, tag="wh")
            for l in range(N_HID):
                nc.sync.dma_start(out=wh_t[:, l * 512:(l + 1) * 512],
                                  in_=wh[e][:, l * 512:(l + 1) * 512])
            bh_t = wpool.tile([128, N_HID * 2], F32, tag="bh")
            nc.sync.dma_start(out=bh_t, in_=bh[e])
            v_t = wpool.tile([128, 2 * 256], F32R, tag="v")
            nc.sync.dma_start(out=v_t, in_=vw[e])

            # ---- layer 0: x^T (7, BC) -> h (2x128, BC); bias folded in ----
            # h is split per b-tile so cross-layer deps are slice-granular
            # (whole-tile deps stall the first matmuls of every layer).
            h_cur = [[hpool.tile([128, 512], F32R, tag=f"h{oc}_{bt}",
                                 name=f"h{oc}_{bt}_e{e}") for bt in range(NT)]
                     for oc in range(2)]
            # bt-major order so the ACT relu queue completes tiles in the
            # order H0's matmuls consume them (H0 (oc, bt) needs both
            # h[0][bt] and h[1][bt])
            for bt in range(NT):
                for oc in range(2):
                    sl = slice(bt * 512, (bt + 1) * 512)
                    ps = pspool.tile([128, 512], F32, tag="ps")
                    nc.tensor.matmul(ps, lhsT=(w0_t[:, oc * 128:(oc + 1) * 128]),
                                     rhs=(x_t[:, sl]), start=True, stop=True)
                    relu_post(h_cur[oc][bt], ps, None, True)

            # ---- 7 hidden layers: K=256 (2 chunks), M=256 (2 chunks) ----
            for l in range(N_HID):
                h_nxt = [[hpool.tile([128, 512], F32R, tag=f"h{oc}_{bt}",
                                     name=f"h{oc}_{bt}_e{e}l{l}")
                          for bt in range(NT)] for oc in range(2)]
                base = l * 2 * N_UNITS
                for oc in range(2):
                    for bt in range(NT):
                        ps = pspool.tile([128, 512], F32, tag="ps")
                        nc.tensor.matmul(
                            ps, lhsT=(wh_t[:, base + oc * 128: base + oc * 128 + 128]),
                            rhs=(h_cur[0][bt]), start=True, stop=False)
                        nc.tensor.matmul(
                            ps, lhsT=(wh_t[:, base + N_UNITS + oc * 128: base + N_UNITS + oc * 128 + 128]),
                            rhs=(h_cur[1][bt]), start=False, stop=True)
                        relu_post(h_nxt[oc][bt], ps,
                                  bh_t[:, l * 2 + oc: l * 2 + oc + 1],
                                  False if l == 0 else (oc * NT + bt) % 2 == 0)
                h_cur = h_nxt

            # ---- final layer: out[o', b] += sum_kc V[e][kc].T @ h[kc] ----
            # Adds are produced one per 2 matmuls (~920 ns) and served on DVE
            # in ~660 ns, so DVE never backs up; the next ensemble's L0 relus
            # go to ACT and never queue behind these. For the last ensemble,
            # each finished output slice DMAs out immediately (73 real rows
            # only for the upper chunk), overlapping the drain.
            for oc in range(2):
                for bt in range(NT):
                    sl = slice(bt * 512, (bt + 1) * 512)
                    ps = pspool.tile([128, 512], F32, tag="ps")
                    nc.tensor.matmul(ps, lhsT=(v_t[:, oc * 128: oc * 128 + 128]),
                                     rhs=(h_cur[0][bt]), start=True, stop=False)
                    nc.tensor.matmul(ps, lhsT=(v_t[:, 256 + oc * 128: 256 + oc * 128 + 128]),
                                     rhs=(h_cur[1][bt]), start=False, stop=True)
                    if e == 0:
                        nc.vector.tensor_scalar(out=out_t[oc][:, sl], in0=ps,
                                                scalar1=bp_t[:, oc:oc + 1],
                                                scalar2=None, op0=add)
                    else:
                        nc.vector.tensor_tensor(out=out_t[oc][:, sl],
                                                in0=out_t[oc][:, sl],
                                                in1=ps, op=add)
                        if e == ENS - 1:
                            if oc == 0:
                                nc.sync.dma_start(out=yt[0:128, sl],
                                                  in_=out_t[0][:, sl])
                            else:
                                nc.scalar.dma_start(out=yt[128:128 + OC1, sl],
                                                    in_=out_t[1][0:OC1, sl])

    nc.compile()
    return nc


def prepare_inputs(x, W0, b0, Wh, bh, Wf, bf):
    """Host-side weight refactoring + per-core sharding. All exact fp32
    linear algebra (bias folds + the channel-mix mean folded into Wf)."""
    x = np.asarray(x, np.float32)
    W0 = np.asarray(W0, np.float32)
    b0 = np.asarray(b0, np.float32)
    Wh = np.asarray(Wh, np.float32)
    bh = np.asarray(bh, np.float32)
    Wf = np.asarray(Wf, np.float32)
    bf = np.asarray(bf, np.float32)

    # layer 0 with bias folded: lhsT rows = 6 inputs + ones row
    w0a = np.concatenate([W0, b0[:, None, :]], axis=1)  # (ENS, 7, 256)
    w0a = np.ascontiguousarray(w0a)

    # hidden weights -> [e, p, (l, kc, o)]
    whh = (Wh.transpose(1, 0, 2, 3)              # (e, l, h, o)
             .reshape(ENS, N_HID, 2, 128, N_UNITS)
             .transpose(0, 3, 1, 2, 4)           # (e, p, l, kc, o)
             .reshape(ENS, 128, N_HID * 2 * N_UNITS))
    whh = np.ascontiguousarray(whh)

    # hidden biases -> [e, p, (l, oc)]
    bhh = (bh.transpose(1, 0, 2)                 # (e, l, o)
             .reshape(ENS, N_HID, 2, 128)
             .transpose(0, 3, 1, 2)              # (e, p, l, oc)
             .reshape(ENS, 128, N_HID * 2))
    bhh = np.ascontiguousarray(bhh)

    # fold the strided channel-mix mean into the final weights:
    # out[b, o'] = 0.1 * sum_j yflat[b, o'*10+j],  yflat col c = e*201+o
    C = ENS * OUT_DIM
    M = np.zeros((C, OUT_DIM), np.float32)
    M[np.arange(C), np.arange(C) // ENS] = 1.0 / ENS
    Me = M.reshape(ENS, OUT_DIM, OUT_DIM)
    V = np.einsum('eho,eoc->ehc', Wf, Me)        # (ENS, 256, 201)
    bpv = bf.reshape(C) @ M                      # (201,)

    Vp = np.zeros((ENS, N_UNITS, 256), np.float32)
    Vp[:, :, :OUT_DIM] = V
    vww = (Vp.reshape(ENS, 2, 128, 256)
             .transpose(0, 2, 1, 3)              # (e, p, kc, o')
             .reshape(ENS, 128, 2 * 256))
    vww = np.ascontiguousarray(vww)

    bp_pad = np.zeros(256, np.float32)
    bp_pad[:OUT_DIM] = bpv
    bp_t = np.ascontiguousarray(bp_pad.reshape(2, 128).T)  # (128, 2)

    ones = np.ones((1, BC), np.float32)
    in_maps = []
    for c in range(N_CORES):
        xs = x[c * BC:(c + 1) * BC]              # (BC, 6)
        xt = np.ascontiguousarray(
            np.concatenate([xs.T, ones], axis=0))  # (7, BC)
        in_maps.append({
            "xt": xt, "w0": w0a, "wh": whh, "bh": bhh, "vw": vww, "bp": bp_t,
        })
    return in_maps


def run(in_maps, trace=False, tmpdir=None):
    if "nc" not in _CACHE:
        _CACHE["nc"] = build_program()
    nc = _CACHE["nc"]
    res = bass_utils.run_bass_kernel_spmd(
        nc, in_maps, core_ids=list(range(N_CORES)), trace=trace, tmpdir=tmpdir)
    return res


def kernel(x, W0, b0, Wh, bh, Wf, bf):
    in_maps = prepare_inputs(x, W0, b0, Wh, bh, Wf, bf)
    res = run(in_maps)
    out = np.empty((BATCH, OUT_DIM), np.float32)
    for c in range(N_CORES):
        out[c * BC:(c + 1) * BC, :] = res.results[c]["yt"][:OUT_DIM].T
    return out



# revision 24
# speedup vs baseline: 1.0195x; 1.0195x over previous
"""Ensemble-MLP (grouped 1x1 conv) Trainium2 kernel.

Computation (per batch row b):
  h = relu(x @ W0[e] + b0[e])             e = 0..9 ensembles, 256 units
  h = relu(h @ Wh[l,e] + bh[l,e])         l = 0..6 hidden layers
  y[e] = h @ Wf[e] + bf[e]                201 outputs per ensemble
  out[b, o'] = mean_j yflat[b, o'*10 + j] (strided channel mix, yflat = e*201+o)

Strategy:
  * Data parallel: batch 16384 -> 2048 rows per core on 8 cores. Weights
    replicated.
  * Activations live in SBUF transposed: H[channel, batch], channel on
    partitions (256 = 2 chunks of 128), batch on the free axis (2048).
  * Every layer is matmul(out[o, b] += W[c, o].T @ H[c, b]) with fp32r
    (full-rate fp32 PE mode, N=512 columns per PSUM bank).
  * Layer-0 bias is folded into the matmul via an all-ones row appended to
    x^T (K=7). Hidden biases ride the relu post-op (per-partition bias).
  * The final channel-mixing mean is folded into the last-layer weights on
    the host: out = sum_e H_e @ V[e] + bp, V[e] = Wf[e] @ Me[e] (exact
    linear algebra, no approximation).
  * relu post-ops alternate between ScalarE (ACT) and VectorE (DVE) so the
    PE stays the bottleneck; final-layer partials accumulate into SBUF.
  * Warmup matmuls on a zeroed tile ramp the PE p-state (0.65 -> 2.4 GHz)
    while the first weight DMAs land, so real matmuls start at full clock.
  * Ensemble-accumulation adds run on the otherwise-idle GpSimd engine
    (e0..e8), keeping ACT/DVE dedicated to relus: no post-op backlog at
    ensemble boundaries means no PE idle gap and no p-state reset.
  * Last ensemble accumulates on DVE/GpSimd and each output slice DMAs
    out as soon as it is final, shrinking the tail drain.
"""

import numpy as np
from contextlib import ExitStack

import concourse.bass as bass
import concourse.mybir as mybir
import concourse.tile as tile
from concourse import bacc, bass_utils

F32 = mybir.dt.float32
F32R = mybir.dt.float32r

ENS, N_UNITS, N_HID, IN_DIM, OUT_DIM, BATCH = 10, 256, 7, 6, 201, 16384
N_CORES = 8
BC = BATCH // N_CORES          # 2048 batch rows per core
NT = BC // 512                 # 4 moving-operand tiles of 512
OC1 = OUT_DIM - 128            # 73 rows in the second output chunk
WARMUP_MM = 8                  # PE p-state warmup matmuls during DMA head

_CACHE = {}


def build_program():
    nc = bacc.Bacc("TRN2", debug=False)

    xt = nc.dram_tensor("xt", (IN_DIM + 1, BC), F32R, kind="ExternalInput").ap()
    w0 = nc.dram_tensor("w0", (ENS, IN_DIM + 1, N_UNITS), F32R, kind="ExternalInput").ap()
    wh = nc.dram_tensor("wh", (ENS, 128, N_HID * 2 * N_UNITS), F32R, kind="ExternalInput").ap()
    bh = nc.dram_tensor("bh", (ENS, 128, N_HID * 2), F32, kind="ExternalInput").ap()
    vw = nc.dram_tensor("vw", (ENS, 128, 2 * 256), F32R, kind="ExternalInput").ap()
    bp = nc.dram_tensor("bp", (128, 2), F32, kind="ExternalInput").ap()
    # (oc, bt)-tiled output layout: every per-slice DMA lands contiguous in
    # DRAM (a strided dst fragments into tiny descriptors on one DMA queue);
    # the host un-tiles. Only OC1=73 rows of the second chunk are real.
    yt = nc.dram_tensor("yt", (2, NT, 128, 512), F32, kind="ExternalOutput").ap()

    add = mybir.AluOpType.add
    mx = mybir.AluOpType.max
    relu = mybir.ActivationFunctionType.Relu

    with ExitStack() as ctx:
        tc = ctx.enter_context(tile.TileContext(nc))
        const = ctx.enter_context(tc.tile_pool(name="const", bufs=1))
        wpool = ctx.enter_context(tc.tile_pool(name="w", bufs=2))
        hpool = ctx.enter_context(tc.tile_pool(name="h", bufs=2))
        opool = ctx.enter_context(tc.tile_pool(name="acc", bufs=1))
        pspool = ctx.enter_context(tc.tile_pool(name="ps", bufs=8, space="PSUM"))

        x_t = const.tile([IN_DIM + 1, BC], F32R)
        bp_t = const.tile([128, 2], F32)
        out_t = [opool.tile([128, BC], F32, tag=f"out{i}", name=f"out{i}")
                 for i in range(2)]

        def relu_post(dst, ps, bias_ap, use_act):
            # oc0 -> ACT, oc1 -> DVE for every layer (4+4 per layer, matching
            # the bt-major production/consumption order), final-layer adds all
            # on DVE. The killer stall this avoids: a relu queued on one
            # engine behind a burst of other work (in-order queues) idles the
            # PE, and any PE idle gap resets its p-state (~10 matmuls at 1.2
            # GHz instead of 2.4 to re-ramp).
            if use_act:
                nc.scalar.activation(out=dst, in_=ps, func=relu,
                                     bias=bias_ap if bias_ap is not None else 0.0)
            elif bias_ap is not None:
                nc.vector.tensor_scalar(out=dst, in0=ps, scalar1=bias_ap,
                                        scalar2=0.0, op0=add, op1=mx)
            else:
                nc.vector.tensor_scalar(out=dst, in0=ps, scalar1=0.0,
                                        scalar2=None, op0=mx)

        for e in range(ENS):
            w0_t = wpool.tile([IN_DIM + 1, N_UNITS], F32R, tag="w0")
            nc.sync.dma_start(out=w0_t, in_=w0[e])
            if e == 0:
                # x lands on a single DMA engine (only 7 partition rows), so
                # issue it right after w0 in b-tile slices: layer-0's first
                # matmuls only wait on their own slice.
                for bt in range(NT):
                    nc.sync.dma_start(out=x_t[:, bt * 512:(bt + 1) * 512],
                                      in_=xt[:, bt * 512:(bt + 1) * 512])
                nc.sync.dma_start(out=bp_t, in_=bp)
                # PE p-state ramps with *continuous* busy time (0.65 -> 1.2
                # -> 2.4 GHz over ~3us of execution). Dummy matmuls on the
                # just-landed w0 tile keep the PE busy through the remaining
                # DMA/queue-bring-up window so real matmuls start at full
                # clock (a cold PE costs ~5us over the first ~25 matmuls).
                for _ in range(WARMUP_MM):
                    ps = pspool.tile([128, 512], F32, tag="ps")
                    nc.tensor.matmul(ps[:, 0:N_UNITS], lhsT=w0_t[:, 0:128],
                                     rhs=w0_t, start=True, stop=True)
            wh_t = wpool.tile([128, N_HID * 2 * N_UNITS], F32R, tag="wh")
            for l in range(N_HID):
                nc.sync.dma_start(out=wh_t[:, l * 512:(l + 1) * 512],
                                  in_=wh[e][:, l * 512:(l + 1) * 512])
            bh_t = wpool.tile([128, N_HID * 2], F32, tag="bh")
            nc.sync.dma_start(out=bh_t, in_=bh[e])
            v_t = wpool.tile([128, 2 * 256], F32R, tag="v")
            nc.sync.dma_start(out=v_t, in_=vw[e])

            # ---- layer 0: x^T (7, BC) -> h (2x128, BC); bias folded in ----
            # h is split per b-tile so cross-layer deps are slice-granular
            # (whole-tile deps stall the first matmuls of every layer).
            h_cur = [[hpool.tile([128, 512], F32R, tag=f"h{oc}_{bt}",
                                 name=f"h{oc}_{bt}_e{e}") for bt in range(NT)]
                     for oc in range(2)]
            # bt-major order everywhere: relus complete in exactly the order
            # the next layer's (bt-major) matmuls consume them
            for bt in range(NT):
                for oc in range(2):
                    sl = slice(bt * 512, (bt + 1) * 512)
                    ps = pspool.tile([128, 512], F32, tag="ps")
                    nc.tensor.matmul(ps, lhsT=(w0_t[:, oc * 128:(oc + 1) * 128]),
                                     rhs=(x_t[:, sl]), start=True, stop=True)
                    relu_post(h_cur[oc][bt], ps, None, oc == 0)

            # ---- 7 hidden layers: K=256 (2 chunks), M=256 (2 chunks) ----
            for l in range(N_HID):
                h_nxt = [[hpool.tile([128, 512], F32R, tag=f"h{oc}_{bt}",
                                     name=f"h{oc}_{bt}_e{e}l{l}")
                          for bt in range(NT)] for oc in range(2)]
                base = l * 2 * N_UNITS
                for bt in range(NT):
                    for oc in range(2):
                        ps = pspool.tile([128, 512], F32, tag="ps")
                        nc.tensor.matmul(
                            ps, lhsT=(wh_t[:, base + oc * 128: base + oc * 128 + 128]),
                            rhs=(h_cur[0][bt]), start=True, stop=False)
                        nc.tensor.matmul(
                            ps, lhsT=(wh_t[:, base + N_UNITS + oc * 128: base + N_UNITS + oc * 128 + 128]),
                            rhs=(h_cur[1][bt]), start=False, stop=True)
                        relu_post(h_nxt[oc][bt], ps,
                                  bh_t[:, l * 2 + oc: l * 2 + oc + 1],
                                  oc == 0)
                h_cur = h_nxt

            # ---- final layer: out[o', b] += sum_kc V[e][kc].T @ h[kc] ----
            # Adds are produced one per 2 matmuls (~920 ns) and served on DVE
            # in ~660 ns, so DVE never backs up; the next ensemble's L0 relus
            # go to ACT and never queue behind these. For the last ensemble,
            # each finished output slice DMAs out immediately (73 real rows
            # only for the upper chunk), overlapping the drain.
            for bt in range(NT):
                for oc in range(2):
                    sl = slice(bt * 512, (bt + 1) * 512)
                    ps = pspool.tile([128, 512], F32, tag="ps")
                    nc.tensor.matmul(ps, lhsT=(v_t[:, oc * 128: oc * 128 + 128]),
                                     rhs=(h_cur[0][bt]), start=True, stop=False)
                    nc.tensor.matmul(ps, lhsT=(v_t[:, 256 + oc * 128: 256 + oc * 128 + 128]),
                                     rhs=(h_cur[1][bt]), start=False, stop=True)
                    if e == 0:
                        nc.vector.tensor_scalar(out=out_t[oc][:, sl], in0=ps,
                                                scalar1=bp_t[:, oc:oc + 1],
                                                scalar2=None, op0=add)
                    else:
                        nc.vector.tensor_tensor(out=out_t[oc][:, sl],
                                                in0=out_t[oc][:, sl],
                                                in1=ps, op=add)
                        if e == ENS - 1:
                            eng = nc.sync if oc == 0 else nc.scalar
                            rows = 128 if oc == 0 else OC1
                            eng.dma_start(out=yt[oc][bt][0:rows],
                                          in_=out_t[oc][0:rows, sl])

    nc.compile()
    return nc


def prepare_inputs(x, W0, b0, Wh, bh, Wf, bf):
    """Host-side weight refactoring + per-core sharding. All exact fp32
    linear algebra (bias folds + the channel-mix mean folded into Wf)."""
    x = np.asarray(x, np.float32)
    W0 = np.asarray(W0, np.float32)
    b0 = np.asarray(b0, np.float32)
    Wh = np.asarray(Wh, np.float32)
    bh = np.asarray(bh, np.float32)
    Wf = np.asarray(Wf, np.float32)
    bf = np.asarray(bf, np.float32)

    # layer 0 with bias folded: lhsT rows = 6 inputs + ones row
    w0a = np.concatenate([W0, b0[:, None, :]], axis=1)  # (ENS, 7, 256)
    w0a = np.ascontiguousarray(w0a)

    # hidden weights -> [e, p, (l, kc, o)]
    whh = (Wh.transpose(1, 0, 2, 3)              # (e, l, h, o)
             .reshape(ENS, N_HID, 2, 128, N_UNITS)
             .transpose(0, 3, 1, 2, 4)           # (e, p, l, kc, o)
             .reshape(ENS, 128, N_HID * 2 * N_UNITS))
    whh = np.ascontiguousarray(whh)

    # hidden biases -> [e, p, (l, oc)]
    bhh = (bh.transpose(1, 0, 2)                 # (e, l, o)
             .reshape(ENS, N_HID, 2, 128)
             .transpose(0, 3, 1, 2)              # (e, p, l, oc)
             .reshape(ENS, 128, N_HID * 2))
    bhh = np.ascontiguousarray(bhh)

    # fold the strided channel-mix mean into the final weights:
    # out[b, o'] = 0.1 * sum_j yflat[b, o'*10+j],  yflat col c = e*201+o
    C = ENS * OUT_DIM
    M = np.zeros((C, OUT_DIM), np.float32)
    M[np.arange(C), np.arange(C) // ENS] = 1.0 / ENS
    Me = M.reshape(ENS, OUT_DIM, OUT_DIM)
    V = np.einsum('eho,eoc->ehc', Wf, Me)        # (ENS, 256, 201)
    bpv = bf.reshape(C) @ M                      # (201,)

    Vp = np.zeros((ENS, N_UNITS, 256), np.float32)
    Vp[:, :, :OUT_DIM] = V
    vww = (Vp.reshape(ENS, 2, 128, 256)
             .transpose(0, 2, 1, 3)              # (e, p, kc, o')
             .reshape(ENS, 128, 2 * 256))
    vww = np.ascontiguousarray(vww)

    bp_pad = np.zeros(256, np.float32)
    bp_pad[:OUT_DIM] = bpv
    bp_t = np.ascontiguousarray(bp_pad.reshape(2, 128).T)  # (128, 2)

    ones = np.ones((1, BC), np.float32)
    in_maps = []
    for c in range(N_CORES):
        xs = x[c * BC:(c + 1) * BC]              # (BC, 6)
        xt = np.ascontiguousarray(
            np.concatenate([xs.T, ones], axis=0))  # (7, BC)
        in_maps.append({
            "xt": xt, "w0": w0a, "wh": whh, "bh": bhh, "vw": vww, "bp": bp_t,
        })
    return in_maps


def run(in_maps, trace=False, tmpdir=None):
    if "nc" not in _CACHE:
        _CACHE["nc"] = build_program()
    nc = _CACHE["nc"]
    res = bass_utils.run_bass_kernel_spmd(
        nc, in_maps, core_ids=list(range(N_CORES)), trace=trace, tmpdir=tmpdir)
    return res


def kernel(x, W0, b0, Wh, bh, Wf, bf):
    in_maps = prepare_inputs(x, W0, b0, Wh, bh, Wf, bf)
    res = run(in_maps)
    out = np.empty((BATCH, OUT_DIM), np.float32)
    for c in range(N_CORES):
        y = res.results[c]["yt"]                 # (2, NT, 128, 512) tiled
        y0 = y[0].transpose(1, 0, 2).reshape(128, BC)
        y1 = y[1].transpose(1, 0, 2).reshape(128, BC)[:OC1]
        out[c * BC:(c + 1) * BC, :] = np.concatenate([y0, y1], axis=0).T
    return out



# revision 28
# speedup vs baseline: 1.0248x; 1.0051x over previous
"""Ensemble-MLP (grouped 1x1 conv) Trainium2 kernel.

Computation (per batch row b):
  h = relu(x @ W0[e] + b0[e])             e = 0..9 ensembles, 256 units
  h = relu(h @ Wh[l,e] + bh[l,e])         l = 0..6 hidden layers
  y[e] = h @ Wf[e] + bf[e]                201 outputs per ensemble
  out[b, o'] = mean_j yflat[b, o'*10 + j] (strided channel mix, yflat = e*201+o)

Strategy:
  * Data parallel: batch 16384 -> 2048 rows per core on 8 cores. Weights
    replicated.
  * Activations live in SBUF transposed: H[channel, batch], channel on
    partitions (256 = 2 chunks of 128), batch on the free axis (2048).
  * Every layer is matmul(out[o, b] += W[c, o].T @ H[c, b]) with fp32r
    (full-rate fp32 PE mode, N=512 columns per PSUM bank).
  * Layer-0 bias is folded into the matmul via an all-ones row appended to
    x^T (K=7). Hidden biases ride the relu post-op (per-partition bias).
  * The final channel-mixing mean is folded into the last-layer weights on
    the host: out = sum_e H_e @ V[e] + bp, V[e] = Wf[e] @ Me[e] (exact
    linear algebra, no approximation).
  * relu post-ops alternate between ScalarE (ACT) and VectorE (DVE) so the
    PE stays the bottleneck; final-layer partials accumulate into SBUF.
  * Warmup matmuls on a zeroed tile ramp the PE p-state (0.65 -> 2.4 GHz)
    while the first weight DMAs land, so real matmuls start at full clock.
  * Ensemble-accumulation adds run on the otherwise-idle GpSimd engine
    (e0..e8), keeping ACT/DVE dedicated to relus: no post-op backlog at
    ensemble boundaries means no PE idle gap and no p-state reset.
  * Last ensemble accumulates on DVE/GpSimd and each output slice DMAs
    out as soon as it is final, shrinking the tail drain.
"""

import numpy as np
from contextlib import ExitStack

import concourse.bass as bass
import concourse.mybir as mybir
import concourse.tile as tile
from concourse import bacc, bass_utils

F32 = mybir.dt.float32
F32R = mybir.dt.float32r

ENS, N_UNITS, N_HID, IN_DIM, OUT_DIM, BATCH = 10, 256, 7, 6, 201, 16384
N_CORES = 8
BC = BATCH // N_CORES          # 2048 batch rows per core
NT = BC // 512                 # 4 moving-operand tiles of 512
OC1 = OUT_DIM - 128            # 73 rows in the second output chunk
WARMUP_MM = 8                  # PE p-state warmup matmuls during DMA head

_CACHE = {}


def build_program():
    nc = bacc.Bacc("TRN2", debug=False)

    xt = nc.dram_tensor("xt", (IN_DIM + 1, BC), F32R, kind="ExternalInput").ap()
    w0 = nc.dram_tensor("w0", (ENS, IN_DIM + 1, N_UNITS), F32R, kind="ExternalInput").ap()
    wh = nc.dram_tensor("wh", (ENS, 128, N_HID * 2 * N_UNITS), F32R, kind="ExternalInput").ap()
    bh = nc.dram_tensor("bh", (ENS, 128, N_HID * 2), F32, kind="ExternalInput").ap()
    vw = nc.dram_tensor("vw", (ENS, 128, 2 * 256), F32R, kind="ExternalInput").ap()
    bp = nc.dram_tensor("bp", (128, 2), F32, kind="ExternalInput").ap()
    yt = nc.dram_tensor("yt", (256, BC), F32, kind="ExternalOutput").ap()

    add = mybir.AluOpType.add
    mx = mybir.AluOpType.max
    relu = mybir.ActivationFunctionType.Relu

    with ExitStack() as ctx:
        tc = ctx.enter_context(tile.TileContext(nc))
        const = ctx.enter_context(tc.tile_pool(name="const", bufs=1))
        wpool = ctx.enter_context(tc.tile_pool(name="w", bufs=2))
        hpool = ctx.enter_context(tc.tile_pool(name="h", bufs=2))
        opool = ctx.enter_context(tc.tile_pool(name="acc", bufs=1))
        pspool = ctx.enter_context(tc.tile_pool(name="ps", bufs=8, space="PSUM"))

        x_t = const.tile([IN_DIM + 1, BC], F32R)
        bp_t = const.tile([128, 2], F32)
        out_t = [opool.tile([128, BC], F32, tag=f"out{i}", name=f"out{i}")
                 for i in range(2)]

        def relu_post(dst, ps, bias_ap, use_act):
            # oc0 -> ACT, oc1 -> DVE for every layer (4+4 per layer, matching
            # the bt-major production/consumption order), final-layer adds all
            # on DVE. The killer stall this avoids: a relu queued on one
            # engine behind a burst of other work (in-order queues) idles the
            # PE, and any PE idle gap resets its p-state (~10 matmuls at 1.2
            # GHz instead of 2.4 to re-ramp).
            if use_act:
                nc.scalar.activation(out=dst, in_=ps, func=relu,
                                     bias=bias_ap if bias_ap is not None else 0.0)
            elif bias_ap is not None:
                nc.vector.tensor_scalar(out=dst, in0=ps, scalar1=bias_ap,
                                        scalar2=0.0, op0=add, op1=mx)
            else:
                nc.vector.tensor_scalar(out=dst, in0=ps, scalar1=0.0,
                                        scalar2=None, op0=mx)

        for e in range(ENS):
            w0_t = wpool.tile([IN_DIM + 1, N_UNITS], F32R, tag="w0")
            nc.sync.dma_start(out=w0_t, in_=w0[e])
            if e == 0:
                # x lands on a single DMA engine (only 7 partition rows), so
                # issue it right after w0 in b-tile slices: layer-0's first
                # matmuls only wait on their own slice.
                for bt in range(NT):
                    nc.sync.dma_start(out=x_t[:, bt * 512:(bt + 1) * 512],
                                      in_=xt[:, bt * 512:(bt + 1) * 512])
                nc.sync.dma_start(out=bp_t, in_=bp)
                # PE p-state ramps with *continuous* busy time (0.65 -> 1.2
                # -> 2.4 GHz over ~3us of execution). Dummy matmuls on the
                # just-landed w0 tile keep the PE busy through the remaining
                # DMA/queue-bring-up window so real matmuls start at full
                # clock (a cold PE costs ~5us over the first ~25 matmuls).
                for _ in range(WARMUP_MM):
                    ps = pspool.tile([128, 512], F32, tag="ps")
                    nc.tensor.matmul(ps[:, 0:N_UNITS], lhsT=w0_t[:, 0:128],
                                     rhs=w0_t, start=True, stop=True)
            # wh layer-slices land on DMA rings with multi-us latency each;
            # split each into halves (parallel rings) and issue in the order
            # compute consumes them, with the H0 bias right after l0 --
            # otherwise e0's H0/H1 stall waiting for weights.
            wh_t = wpool.tile([128, N_HID * 2 * N_UNITS], F32R, tag="wh")
            bh_t = wpool.tile([128, N_HID * 2], F32, tag="bh")
            v_t = wpool.tile([128, 2 * 256], F32R, tag="v")
            for l in range(N_HID):
                for hf in range(2):
                    s0 = l * 512 + hf * 256
                    nc.sync.dma_start(out=wh_t[:, s0:s0 + 256],
                                      in_=wh[e][:, s0:s0 + 256])
                if l == 0:
                    nc.sync.dma_start(out=bh_t, in_=bh[e])
            for hf in range(2):
                nc.sync.dma_start(out=v_t[:, hf * 256:(hf + 1) * 256],
                                  in_=vw[e][:, hf * 256:(hf + 1) * 256])

            # ---- layer 0: x^T (7, BC) -> h (2x128, BC); bias folded in ----
            # h is split per b-tile so cross-layer deps are slice-granular
            # (whole-tile deps stall the first matmuls of every layer).
            h_cur = [[hpool.tile([128, 512], F32R, tag=f"h{oc}_{bt}",
                                 name=f"h{oc}_{bt}_e{e}") for bt in range(NT)]
                     for oc in range(2)]
            # bt-major order everywhere: relus complete in exactly the order
            # the next layer's (bt-major) matmuls consume them
            for bt in range(NT):
                for oc in range(2):
                    sl = slice(bt * 512, (bt + 1) * 512)
                    ps = pspool.tile([128, 512], F32, tag="ps")
                    nc.tensor.matmul(ps, lhsT=(w0_t[:, oc * 128:(oc + 1) * 128]),
                                     rhs=(x_t[:, sl]), start=True, stop=True)
                    relu_post(h_cur[oc][bt], ps, None, oc == 0)

            # ---- 7 hidden layers: K=256 (2 chunks), M=256 (2 chunks) ----
            for l in range(N_HID):
                h_nxt = [[hpool.tile([128, 512], F32R, tag=f"h{oc}_{bt}",
                                     name=f"h{oc}_{bt}_e{e}l{l}")
                          for bt in range(NT)] for oc in range(2)]
                base = l * 2 * N_UNITS
                for bt in range(NT):
                    for oc in range(2):
                        ps = pspool.tile([128, 512], F32, tag="ps")
                        nc.tensor.matmul(
                            ps, lhsT=(wh_t[:, base + oc * 128: base + oc * 128 + 128]),
                            rhs=(h_cur[0][bt]), start=True, stop=False)
                        nc.tensor.matmul(
                            ps, lhsT=(wh_t[:, base + N_UNITS + oc * 128: base + N_UNITS + oc * 128 + 128]),
                            rhs=(h_cur[1][bt]), start=False, stop=True)
                        relu_post(h_nxt[oc][bt], ps,
                                  bh_t[:, l * 2 + oc: l * 2 + oc + 1],
                                  oc == 0)
                h_cur = h_nxt

            # ---- final layer: out[o', b] += sum_kc V[e][kc].T @ h[kc] ----
            # Adds are produced one per 2 matmuls (~920 ns) and served on DVE
            # in ~660 ns, so DVE never backs up; the next ensemble's L0 relus
            # go to ACT and never queue behind these. For the last ensemble,
            # each finished output slice DMAs out immediately (73 real rows
            # only for the upper chunk), overlapping the drain.
            # Last ensemble runs oc-major so out_t[0] is final mid-phase and
            # its 1MB DMA overlaps the oc1 matmuls; out_t[1] moves only the
            # OC1=73 real rows. Both DMAs are contiguous in DRAM -- a strided
            # or small dst fragments onto a single DMA ring (~10us/MB).
            last = e == ENS - 1
            order = ([(0, bt) for bt in range(NT)] + [(1, bt) for bt in range(NT)]
                     ) if last else [(oc, bt) for bt in range(NT) for oc in range(2)]
            for oc, bt in order:
                    sl = slice(bt * 512, (bt + 1) * 512)
                    ps = pspool.tile([128, 512], F32, tag="ps")
                    nc.tensor.matmul(ps, lhsT=(v_t[:, oc * 128: oc * 128 + 128]),
                                     rhs=(h_cur[0][bt]), start=True, stop=False)
                    nc.tensor.matmul(ps, lhsT=(v_t[:, 256 + oc * 128: 256 + oc * 128 + 128]),
                                     rhs=(h_cur[1][bt]), start=False, stop=True)
                    if e == 0:
                        nc.vector.tensor_scalar(out=out_t[oc][:, sl], in0=ps,
                                                scalar1=bp_t[:, oc:oc + 1],
                                                scalar2=None, op0=add)
                    else:
                        nc.vector.tensor_tensor(out=out_t[oc][:, sl],
                                                in0=out_t[oc][:, sl],
                                                in1=ps, op=add)
                        if last and bt == NT - 1:
                            if oc == 0:
                                nc.sync.dma_start(out=yt[0:128, :], in_=out_t[0])
                            else:
                                nc.scalar.dma_start(out=yt[128:128 + OC1, :],
                                                    in_=out_t[1][0:OC1, :])

    nc.compile()
    return nc


def prepare_inputs(x, W0, b0, Wh, bh, Wf, bf):
    """Host-side weight refactoring + per-core sharding. All exact fp32
    linear algebra (bias folds + the channel-mix mean folded into Wf)."""
    x = np.asarray(x, np.float32)
    W0 = np.asarray(W0, np.float32)
    b0 = np.asarray(b0, np.float32)
    Wh = np.asarray(Wh, np.float32)
    bh = np.asarray(bh, np.float32)
    Wf = np.asarray(Wf, np.float32)
    bf = np.asarray(bf, np.float32)

    # layer 0 with bias folded: lhsT rows = 6 inputs + ones row
    w0a = np.concatenate([W0, b0[:, None, :]], axis=1)  # (ENS, 7, 256)
    w0a = np.ascontiguousarray(w0a)

    # hidden weights -> [e, p, (l, kc, o)]
    whh = (Wh.transpose(1, 0, 2, 3)              # (e, l, h, o)
             .reshape(ENS, N_HID, 2, 128, N_UNITS)
             .transpose(0, 3, 1, 2, 4)           # (e, p, l, kc, o)
             .reshape(ENS, 128, N_HID * 2 * N_UNITS))
    whh = np.ascontiguousarray(whh)

    # hidden biases -> [e, p, (l, oc)]
    bhh = (bh.transpose(1, 0, 2)                 # (e, l, o)
             .reshape(ENS, N_HID, 2, 128)
             .transpose(0, 3, 1, 2)              # (e, p, l, oc)
             .reshape(ENS, 128, N_HID * 2))
    bhh = np.ascontiguousarray(bhh)

    # fold the strided channel-mix mean into the final weights:
    # out[b, o'] = 0.1 * sum_j yflat[b, o'*10+j],  yflat col c = e*201+o
    C = ENS * OUT_DIM
    M = np.zeros((C, OUT_DIM), np.float32)
    M[np.arange(C), np.arange(C) // ENS] = 1.0 / ENS
    Me = M.reshape(ENS, OUT_DIM, OUT_DIM)
    V = np.einsum('eho,eoc->ehc', Wf, Me)        # (ENS, 256, 201)
    bpv = bf.reshape(C) @ M                      # (201,)

    Vp = np.zeros((ENS, N_UNITS, 256), np.float32)
    Vp[:, :, :OUT_DIM] = V
    vww = (Vp.reshape(ENS, 2, 128, 256)
             .transpose(0, 2, 1, 3)              # (e, p, kc, o')
             .reshape(ENS, 128, 2 * 256))
    vww = np.ascontiguousarray(vww)

    bp_pad = np.zeros(256, np.float32)
    bp_pad[:OUT_DIM] = bpv
    bp_t = np.ascontiguousarray(bp_pad.reshape(2, 128).T)  # (128, 2)

    ones = np.ones((1, BC), np.float32)
    in_maps = []
    for c in range(N_CORES):
        xs = x[c * BC:(c + 1) * BC]              # (BC, 6)
        xt = np.ascontiguousarray(
            np.concatenate([xs.T, ones], axis=0))  # (7, BC)
        in_maps.append({
            "xt": xt, "w0": w0a, "wh": whh, "bh": bhh, "vw": vww, "bp": bp_t,
        })
    return in_maps


def run(in_maps, trace=False, tmpdir=None):
    if "nc" not in _CACHE:
        _CACHE["nc"] = build_program()
    nc = _CACHE["nc"]
    res = bass_utils.run_bass_kernel_spmd(
        nc, in_maps, core_ids=list(range(N_CORES)), trace=trace, tmpdir=tmpdir)
    return res


def kernel(x, W0, b0, Wh, bh, Wf, bf):
    in_maps = prepare_inputs(x, W0, b0, Wh, bh, Wf, bf)
    res = run(in_maps)
    out = np.empty((BATCH, OUT_DIM), np.float32)
    for c in range(N_CORES):
        out[c * BC:(c + 1) * BC, :] = res.results[c]["yt"][:OUT_DIM].T
    return out



# revision 30
# speedup vs baseline: 1.0546x; 1.0291x over previous
"""Ensemble-MLP (grouped 1x1 conv) Trainium2 kernel.

Computation (per batch row b):
  h = relu(x @ W0[e] + b0[e])             e = 0..9 ensembles, 256 units
  h = relu(h @ Wh[l,e] + bh[l,e])         l = 0..6 hidden layers
  y[e] = h @ Wf[e] + bf[e]                201 outputs per ensemble
  out[b, o'] = mean_j yflat[b, o'*10 + j] (strided channel mix, yflat = e*201+o)

Strategy:
  * Data parallel: batch 16384 -> 2048 rows per core on 8 cores. Weights
    replicated.
  * Activations live in SBUF transposed: H[channel, batch], channel on
    partitions (256 = 2 chunks of 128), batch on the free axis (2048).
  * Every layer is matmul(out[o, b] += W[c, o].T @ H[c, b]) with fp32r
    (full-rate fp32 PE mode, N=512 columns per PSUM bank).
  * Layer-0 bias is folded into the matmul via an all-ones row appended to
    x^T (K=7). Hidden biases ride the relu post-op (per-partition bias).
  * The final channel-mixing mean is folded into the last-layer weights on
    the host: out = sum_e H_e @ V[e] + bp, V[e] = Wf[e] @ Me[e] (exact
    linear algebra, no approximation).
  * relu post-ops alternate between ScalarE (ACT) and VectorE (DVE) so the
    PE stays the bottleneck; final-layer partials accumulate into SBUF.
  * Warmup matmuls on a zeroed tile ramp the PE p-state (0.65 -> 2.4 GHz)
    while the first weight DMAs land, so real matmuls start at full clock.
  * Ensemble-accumulation adds run on the otherwise-idle GpSimd engine
    (e0..e8), keeping ACT/DVE dedicated to relus: no post-op backlog at
    ensemble boundaries means no PE idle gap and no p-state reset.
  * Last ensemble accumulates on DVE/GpSimd and each output slice DMAs
    out as soon as it is final, shrinking the tail drain.
"""

import numpy as np
from contextlib import ExitStack

import concourse.bass as bass
import concourse.mybir as mybir
import concourse.tile as tile
from concourse import bacc, bass_utils

F32 = mybir.dt.float32
F32R = mybir.dt.float32r

ENS, N_UNITS, N_HID, IN_DIM, OUT_DIM, BATCH = 10, 256, 7, 6, 201, 16384
N_CORES = 8
BC = BATCH // N_CORES          # 2048 batch rows per core
NT = BC // 512                 # 4 moving-operand tiles of 512
OC1 = OUT_DIM - 128            # 73 rows in the second output chunk
WARMUP_MM = 8                  # PE p-state warmup matmuls during DMA head

_CACHE = {}


def build_program():
    nc = bacc.Bacc("TRN2", debug=False)

    xt = nc.dram_tensor("xt", (IN_DIM + 1, BC), F32R, kind="ExternalInput").ap()
    w0 = nc.dram_tensor("w0", (ENS, IN_DIM + 1, N_UNITS), F32R, kind="ExternalInput").ap()
    wh = nc.dram_tensor("wh", (ENS, 128, N_HID * 2 * N_UNITS), F32R, kind="ExternalInput").ap()
    bh = nc.dram_tensor("bh", (ENS, 128, N_HID * 2), F32, kind="ExternalInput").ap()
    vw = nc.dram_tensor("vw", (ENS, 128, 2 * 256), F32R, kind="ExternalInput").ap()
    bp = nc.dram_tensor("bp", (128, 2), F32, kind="ExternalInput").ap()
    yt = nc.dram_tensor("yt", (256, BC), F32, kind="ExternalOutput").ap()

    add = mybir.AluOpType.add
    mx = mybir.AluOpType.max
    relu = mybir.ActivationFunctionType.Relu

    with ExitStack() as ctx:
        tc = ctx.enter_context(tile.TileContext(nc))
        const = ctx.enter_context(tc.tile_pool(name="const", bufs=1))
        wpool = ctx.enter_context(tc.tile_pool(name="w", bufs=2))
        hpool = ctx.enter_context(tc.tile_pool(name="h", bufs=2))
        opool = ctx.enter_context(tc.tile_pool(name="acc", bufs=1))
        pspool = ctx.enter_context(tc.tile_pool(name="ps", bufs=8, space="PSUM"))

        x_t = const.tile([IN_DIM + 1, BC], F32R)
        bp_t = const.tile([128, 2], F32)
        out_t = [opool.tile([128, BC], F32, tag=f"out{i}", name=f"out{i}")
                 for i in range(2)]

        def relu_post(dst, ps, bias_ap, use_act):
            # oc0 -> ACT, oc1 -> DVE for every layer (4+4 per layer, matching
            # the bt-major production/consumption order), final-layer adds all
            # on DVE. The killer stall this avoids: a relu queued on one
            # engine behind a burst of other work (in-order queues) idles the
            # PE, and any PE idle gap resets its p-state (~10 matmuls at 1.2
            # GHz instead of 2.4 to re-ramp).
            if use_act:
                nc.scalar.activation(out=dst, in_=ps, func=relu,
                                     bias=bias_ap if bias_ap is not None else 0.0)
            elif bias_ap is not None:
                nc.vector.tensor_scalar(out=dst, in0=ps, scalar1=bias_ap,
                                        scalar2=0.0, op0=add, op1=mx)
            else:
                nc.vector.tensor_scalar(out=dst, in0=ps, scalar1=0.0,
                                        scalar2=None, op0=mx)

        for e in range(ENS):
            w0_t = wpool.tile([IN_DIM + 1, N_UNITS], F32R, tag="w0")
            nc.sync.dma_start(out=w0_t, in_=w0[e])
            if e == 0:
                # x lands on a single DMA engine (only 7 partition rows), so
                # issue it right after w0 in b-tile slices: layer-0's first
                # matmuls only wait on their own slice.
                for bt in range(NT):
                    nc.sync.dma_start(out=x_t[:, bt * 512:(bt + 1) * 512],
                                      in_=xt[:, bt * 512:(bt + 1) * 512])
                nc.sync.dma_start(out=bp_t, in_=bp)
                # PE p-state ramps with *continuous* busy time (0.65 -> 1.2
                # -> 2.4 GHz over ~3us of execution). Dummy matmuls on the
                # just-landed w0 tile keep the PE busy through the remaining
                # DMA/queue-bring-up window so real matmuls start at full
                # clock (a cold PE costs ~5us over the first ~25 matmuls).
                for _ in range(WARMUP_MM):
                    ps = pspool.tile([128, 512], F32, tag="ps")
                    nc.tensor.matmul(ps[:, 0:N_UNITS], lhsT=w0_t[:, 0:128],
                                     rhs=w0_t, start=True, stop=True)
            # Each dma_start lands on ~one DMA ring at ~26 GB/s (2KB lines),
            # different dma_starts run on rings in parallel. Issue in the
            # order compute consumes (l0, H0 bias, l1, ...); for e0 only,
            # split l0/l1 into column chunks across several rings so H0/H1
            # don't wait out a full 256KB single-ring transfer.
            wh_t = wpool.tile([128, N_HID * 2 * N_UNITS], F32R, tag="wh")
            bh_t = wpool.tile([128, N_HID * 2], F32, tag="bh")
            v_t = wpool.tile([128, 2 * 256], F32R, tag="v")
            nsplit = {0: 4, 1: 2} if e == 0 else {}
            for l in range(N_HID):
                ns = nsplit.get(l, 1)
                for hf in range(ns):
                    w = 512 // ns
                    s0 = l * 512 + hf * w
                    nc.sync.dma_start(out=wh_t[:, s0:s0 + w],
                                      in_=wh[e][:, s0:s0 + w])
                if l == 0:
                    nc.sync.dma_start(out=bh_t, in_=bh[e])
            nc.sync.dma_start(out=v_t, in_=vw[e])

            # ---- layer 0: x^T (7, BC) -> h (2x128, BC); bias folded in ----
            # h is split per b-tile so cross-layer deps are slice-granular
            # (whole-tile deps stall the first matmuls of every layer).
            h_cur = [[hpool.tile([128, 512], F32R, tag=f"h{oc}_{bt}",
                                 name=f"h{oc}_{bt}_e{e}") for bt in range(NT)]
                     for oc in range(2)]
            # bt-major order everywhere: relus complete in exactly the order
            # the next layer's (bt-major) matmuls consume them
            for bt in range(NT):
                for oc in range(2):
                    sl = slice(bt * 512, (bt + 1) * 512)
                    ps = pspool.tile([128, 512], F32, tag="ps")
                    nc.tensor.matmul(ps, lhsT=(w0_t[:, oc * 128:(oc + 1) * 128]),
                                     rhs=(x_t[:, sl]), start=True, stop=True)
                    relu_post(h_cur[oc][bt], ps, None, oc == 0)

            # ---- 7 hidden layers: K=256 (2 chunks), M=256 (2 chunks) ----
            for l in range(N_HID):
                h_nxt = [[hpool.tile([128, 512], F32R, tag=f"h{oc}_{bt}",
                                     name=f"h{oc}_{bt}_e{e}l{l}")
                          for bt in range(NT)] for oc in range(2)]
                base = l * 2 * N_UNITS
                for bt in range(NT):
                    for oc in range(2):
                        ps = pspool.tile([128, 512], F32, tag="ps")
                        nc.tensor.matmul(
                            ps, lhsT=(wh_t[:, base + oc * 128: base + oc * 128 + 128]),
                            rhs=(h_cur[0][bt]), start=True, stop=False)
                        nc.tensor.matmul(
                            ps, lhsT=(wh_t[:, base + N_UNITS + oc * 128: base + N_UNITS + oc * 128 + 128]),
                            rhs=(h_cur[1][bt]), start=False, stop=True)
                        relu_post(h_nxt[oc][bt], ps,
                                  bh_t[:, l * 2 + oc: l * 2 + oc + 1],
                                  oc == 0)
                h_cur = h_nxt

            # ---- final layer: out[o', b] += sum_kc V[e][kc].T @ h[kc] ----
            # Adds are produced one per 2 matmuls (~920 ns) and served on DVE
            # in ~660 ns, so DVE never backs up; the next ensemble's L0 relus
            # go to ACT and never queue behind these. For the last ensemble,
            # each finished output slice DMAs out immediately (73 real rows
            # only for the upper chunk), overlapping the drain.
            # Last ensemble runs oc-major so out_t[0] is final mid-phase and
            # its 1MB DMA overlaps the oc1 matmuls; out_t[1] moves only the
            # OC1=73 real rows. Both DMAs are contiguous in DRAM -- a strided
            # or small dst fragments onto a single DMA ring (~10us/MB).
            last = e == ENS - 1
            order = ([(0, bt) for bt in range(NT)] + [(1, bt) for bt in range(NT)]
                     ) if last else [(oc, bt) for bt in range(NT) for oc in range(2)]
            for oc, bt in order:
                    sl = slice(bt * 512, (bt + 1) * 512)
                    ps = pspool.tile([128, 512], F32, tag="ps")
                    nc.tensor.matmul(ps, lhsT=(v_t[:, oc * 128: oc * 128 + 128]),
                                     rhs=(h_cur[0][bt]), start=True, stop=False)
                    nc.tensor.matmul(ps, lhsT=(v_t[:, 256 + oc * 128: 256 + oc * 128 + 128]),
                                     rhs=(h_cur[1][bt]), start=False, stop=True)
                    if e == 0:
                        nc.vector.tensor_scalar(out=out_t[oc][:, sl], in0=ps,
                                                scalar1=bp_t[:, oc:oc + 1],
                                                scalar2=None, op0=add)
                    else:
                        nc.vector.tensor_tensor(out=out_t[oc][:, sl],
                                                in0=out_t[oc][:, sl],
                                                in1=ps, op=add)
                        if last and bt == NT - 1:
                            # stripe the output DMA: one dma_start per 8 rows
                            # so the transfer spreads across DMA rings
                            # (contiguous 64KB each) instead of serializing
                            # ~26 GB/s on a single ring
                            rows = 128 if oc == 0 else OC1
                            for r0 in range(0, rows, 8):
                                r1 = min(r0 + 8, rows)
                                eng = nc.sync if (r0 // 8) % 2 == 0 else nc.scalar
                                eng.dma_start(
                                    out=yt[oc * 128 + r0: oc * 128 + r1, :],
                                    in_=out_t[oc][r0:r1, :])

    nc.compile()
    return nc


def prepare_inputs(x, W0, b0, Wh, bh, Wf, bf):
    """Host-side weight refactoring + per-core sharding. All exact fp32
    linear algebra (bias folds + the channel-mix mean folded into Wf)."""
    x = np.asarray(x, np.float32)
    W0 = np.asarray(W0, np.float32)
    b0 = np.asarray(b0, np.float32)
    Wh = np.asarray(Wh, np.float32)
    bh = np.asarray(bh, np.float32)
    Wf = np.asarray(Wf, np.float32)
    bf = np.asarray(bf, np.float32)

    # layer 0 with bias folded: lhsT rows = 6 inputs + ones row
    w0a = np.concatenate([W0, b0[:, None, :]], axis=1)  # (ENS, 7, 256)
    w0a = np.ascontiguousarray(w0a)

    # hidden weights -> [e, p, (l, kc, o)]
    whh = (Wh.transpose(1, 0, 2, 3)              # (e, l, h, o)
             .reshape(ENS, N_HID, 2, 128, N_UNITS)
             .transpose(0, 3, 1, 2, 4)           # (e, p, l, kc, o)
             .reshape(ENS, 128, N_HID * 2 * N_UNITS))
    whh = np.ascontiguousarray(whh)

    # hidden biases -> [e, p, (l, oc)]
    bhh = (bh.transpose(1, 0, 2)                 # (e, l, o)
             .reshape(ENS, N_HID, 2, 128)
             .transpose(0, 3, 1, 2)              # (e, p, l, oc)
             .reshape(ENS, 128, N_HID * 2))
    bhh = np.ascontiguousarray(bhh)

    # fold the strided channel-mix mean into the final weights:
    # out[b, o'] = 0.1 * sum_j yflat[b, o'*10+j],  yflat col c = e*201+o
    C = ENS * OUT_DIM
    M = np.zeros((C, OUT_DIM), np.float32)
    M[np.arange(C), np.arange(C) // ENS] = 1.0 / ENS
    Me = M.reshape(ENS, OUT_DIM, OUT_DIM)
    V = np.einsum('eho,eoc->ehc', Wf, Me)        # (ENS, 256, 201)
    bpv = bf.reshape(C) @ M                      # (201,)

    Vp = np.zeros((ENS, N_UNITS, 256), np.float32)
    Vp[:, :, :OUT_DIM] = V
    vww = (Vp.reshape(ENS, 2, 128, 256)
             .transpose(0, 2, 1, 3)              # (e, p, kc, o')
             .reshape(ENS, 128, 2 * 256))
    vww = np.ascontiguousarray(vww)

    bp_pad = np.zeros(256, np.float32)
    bp_pad[:OUT_DIM] = bpv
    bp_t = np.ascontiguousarray(bp_pad.reshape(2, 128).T)  # (128, 2)

    ones = np.ones((1, BC), np.float32)
    in_maps = []
    for c in range(N_CORES):
        xs = x[c * BC:(c + 1) * BC]              # (BC, 6)
        xt = np.ascontiguousarray(
            np.concatenate([xs.T, ones], axis=0))  # (7, BC)
        in_maps.append({
            "xt": xt, "w0": w0a, "wh": whh, "bh": bhh, "vw": vww, "bp": bp_t,
        })
    return in_maps


def run(in_maps, trace=False, tmpdir=None):
    if "nc" not in _CACHE:
        _CACHE["nc"] = build_program()
    nc = _CACHE["nc"]
    res = bass_utils.run_bass_kernel_spmd(
        nc, in_maps, core_ids=list(range(N_CORES)), trace=trace, tmpdir=tmpdir)
    return res


def kernel(x, W0, b0, Wh, bh, Wf, bf):
    in_maps = prepare_inputs(x, W0, b0, Wh, bh, Wf, bf)
    res = run(in_maps)
    out = np.empty((BATCH, OUT_DIM), np.float32)
    for c in range(N_CORES):
        out[c * BC:(c + 1) * BC, :] = res.results[c]["yt"][:OUT_DIM].T
    return out



# revision 37
# speedup vs baseline: 1.0661x; 1.0108x over previous
"""Ensemble-MLP (grouped 1x1 conv) Trainium2 kernel.

Computation (per batch row b):
  h = relu(x @ W0[e] + b0[e])             e = 0..9 ensembles, 256 units
  h = relu(h @ Wh[l,e] + bh[l,e])         l = 0..6 hidden layers
  y[e] = h @ Wf[e] + bf[e]                201 outputs per ensemble
  out[b, o'] = mean_j yflat[b, o'*10 + j] (strided channel mix, yflat = e*201+o)

Strategy:
  * Data parallel: batch 16384 -> 2048 rows per core on 8 cores. Weights
    replicated.
  * Activations live in SBUF transposed: H[channel, batch], channel on
    partitions (256 = 2 chunks of 128), batch on the free axis (2048).
  * Every layer is matmul(out[o, b] += W[c, o].T @ H[c, b]) with fp32r
    (full-rate fp32 PE mode, N=512 columns per PSUM bank).
  * Layer-0 bias is folded into the matmul via an all-ones row appended to
    x^T (K=7). Hidden biases ride the relu post-op (per-partition bias).
  * The final channel-mixing mean is folded into the last-layer weights on
    the host: out = sum_e H_e @ V[e] + bp, V[e] = Wf[e] @ Me[e] (exact
    linear algebra, no approximation).
  * relu post-ops alternate between ScalarE (ACT) and VectorE (DVE) so the
    PE stays the bottleneck; final-layer partials accumulate into SBUF.
  * Warmup matmuls on a zeroed tile ramp the PE p-state (0.65 -> 2.4 GHz)
    while the first weight DMAs land, so real matmuls start at full clock.
  * Ensemble-accumulation adds run on the otherwise-idle GpSimd engine
    (e0..e8), keeping ACT/DVE dedicated to relus: no post-op backlog at
    ensemble boundaries means no PE idle gap and no p-state reset.
  * Last ensemble accumulates on DVE/GpSimd and each output slice DMAs
    out as soon as it is final, shrinking the tail drain.
"""

import numpy as np
from contextlib import ExitStack

import concourse.bass as bass
import concourse.mybir as mybir
import concourse.tile as tile
from concourse import bacc, bass_utils

F32 = mybir.dt.float32
F32R = mybir.dt.float32r
BF16 = mybir.dt.bfloat16

ENS, N_UNITS, N_HID, IN_DIM, OUT_DIM, BATCH = 10, 256, 7, 6, 201, 16384
N_CORES = 8
BC = BATCH // N_CORES          # 2048 batch rows per core
NT = BC // 512                 # 4 moving-operand tiles of 512
OC1 = OUT_DIM - 128            # 73 rows in the second output chunk
WARMUP_MM = 8                  # PE p-state warmup matmuls during DMA head

_CACHE = {}


def build_program():
    nc = bacc.Bacc("TRN2", debug=False)

    xt = nc.dram_tensor("xt", (IN_DIM + 1, BC), F32R, kind="ExternalInput").ap()
    w0 = nc.dram_tensor("w0", (ENS, IN_DIM + 1, N_UNITS), F32R, kind="ExternalInput").ap()
    wh = nc.dram_tensor("wh", (ENS, 128, N_HID * 2 * N_UNITS), F32R, kind="ExternalInput").ap()
    bh = nc.dram_tensor("bh", (ENS, 128, N_HID * 2), F32, kind="ExternalInput").ap()
    vw = nc.dram_tensor("vw", (ENS, 128, 2 * 256), F32R, kind="ExternalInput").ap()
    bp = nc.dram_tensor("bp", (128, 2), F32, kind="ExternalInput").ap()
    # output leaves the chip in bf16 (halves the critical-path DMA; the
    # ~0.2% quantization of the final value is far inside the error budget)
    yt = nc.dram_tensor("yt", (256, BC), BF16, kind="ExternalOutput").ap()

    add = mybir.AluOpType.add
    mx = mybir.AluOpType.max
    relu = mybir.ActivationFunctionType.Relu

    with ExitStack() as ctx:
        tc = ctx.enter_context(tile.TileContext(nc))
        const = ctx.enter_context(tc.tile_pool(name="const", bufs=1))
        wpool = ctx.enter_context(tc.tile_pool(name="w", bufs=2))
        hpool = ctx.enter_context(tc.tile_pool(name="h", bufs=2))
        opool = ctx.enter_context(tc.tile_pool(name="acc", bufs=1))
        pspool = ctx.enter_context(tc.tile_pool(name="ps", bufs=8, space="PSUM"))

        x_t = const.tile([IN_DIM + 1, BC], F32R)
        bp_t = const.tile([128, 2], F32)
        out_t = [opool.tile([128, BC], F32, tag=f"out{i}", name=f"out{i}")
                 for i in range(2)]
        outb_t = [opool.tile([128, BC], BF16, tag=f"outb{i}", name=f"outb{i}")
                  for i in range(2)]

        def relu_post(dst, ps, bias_ap, use_act):
            # oc0 -> ACT, oc1 -> DVE for every layer (4+4 per layer, matching
            # the bt-major production/consumption order), final-layer adds all
            # on DVE. The killer stall this avoids: a relu queued on one
            # engine behind a burst of other work (in-order queues) idles the
            # PE, and any PE idle gap resets its p-state (~10 matmuls at 1.2
            # GHz instead of 2.4 to re-ramp).
            if use_act:
                nc.scalar.activation(out=dst, in_=ps, func=relu,
                                     bias=bias_ap if bias_ap is not None else 0.0)
            elif bias_ap is not None:
                nc.vector.tensor_scalar(out=dst, in0=ps, scalar1=bias_ap,
                                        scalar2=0.0, op0=add, op1=mx)
            else:
                nc.vector.tensor_scalar(out=dst, in0=ps, scalar1=0.0,
                                        scalar2=None, op0=mx)

        for e in range(ENS):
            w0_t = wpool.tile([IN_DIM + 1, N_UNITS], F32R, tag="w0")
            nc.sync.dma_start(out=w0_t, in_=w0[e])
            if e == 0:
                # x lands on a single DMA engine (only 7 partition rows), so
                # issue it right after w0 in b-tile slices: layer-0's first
                # matmuls only wait on their own slice.
                for bt in range(NT):
                    nc.sync.dma_start(out=x_t[:, bt * 512:(bt + 1) * 512],
                                      in_=xt[:, bt * 512:(bt + 1) * 512])
                nc.sync.dma_start(out=bp_t, in_=bp)
                # PE p-state ramps with *continuous* busy time (0.65 -> 1.2
                # -> 2.4 GHz over ~3us of execution). Dummy matmuls on the
                # just-landed w0 tile keep the PE busy through the remaining
                # DMA/queue-bring-up window so real matmuls start at full
                # clock (a cold PE costs ~5us over the first ~25 matmuls).
                for _ in range(WARMUP_MM):
                    ps = pspool.tile([128, 512], F32, tag="ps")
                    nc.tensor.matmul(ps[:, 0:N_UNITS], lhsT=w0_t[:, 0:128],
                                     rhs=w0_t, start=True, stop=True)
            # Each dma_start lands on ~one DMA ring at ~26 GB/s (2KB lines),
            # different dma_starts run on rings in parallel. Issue in the
            # order compute consumes (l0, H0 bias, l1, ...); for e0 only,
            # split l0/l1 into column chunks across several rings so H0/H1
            # don't wait out a full 256KB single-ring transfer.
            # wh/v use bufs=1: subtile WAR deps gate each layer-slice DMA on
            # THIS ensemble's own H_l reads, so e+1's weights stream in
            # phase-locked just-in-time (~26us lead, ~10us transfer) instead
            # of bursting 2MB at e+1's L0/H0 and slowing the PE's SBUF reads.
            wh_t = wpool.tile([128, N_HID * 2 * N_UNITS], F32R, tag="wh", bufs=1)
            bh_t = wpool.tile([128, N_HID * 2], F32, tag="bh")
            v_t = wpool.tile([128, 2 * 256], F32R, tag="v", bufs=1)
            nsplit = {0: 4, 1: 2} if e == 0 else {}
            for l in range(N_HID):
                ns = nsplit.get(l, 1)
                for hf in range(ns):
                    w = 512 // ns
                    s0 = l * 512 + hf * w
                    nc.sync.dma_start(out=wh_t[:, s0:s0 + w],
                                      in_=wh[e][:, s0:s0 + w])
                if l == 0:
                    nc.sync.dma_start(out=bh_t, in_=bh[e])
            nc.sync.dma_start(out=v_t, in_=vw[e])

            # ---- layer 0: x^T (7, BC) -> h (2x128, BC); bias folded in ----
            # h is split per b-tile so cross-layer deps are slice-granular
            # (whole-tile deps stall the first matmuls of every layer).
            h_cur = [[hpool.tile([128, 512], F32R, tag=f"h{oc}_{bt}",
                                 name=f"h{oc}_{bt}_e{e}") for bt in range(NT)]
                     for oc in range(2)]
            # bt-major order everywhere: relus complete in exactly the order
            # the next layer's (bt-major) matmuls consume them
            for bt in range(NT):
                for oc in range(2):
                    sl = slice(bt * 512, (bt + 1) * 512)
                    ps = pspool.tile([128, 512], F32, tag="ps")
                    nc.tensor.matmul(ps, lhsT=(w0_t[:, oc * 128:(oc + 1) * 128]),
                                     rhs=(x_t[:, sl]), start=True, stop=True)
                    relu_post(h_cur[oc][bt], ps, None, oc == 0)

            # ---- 7 hidden layers: K=256 (2 chunks), M=256 (2 chunks) ----
            for l in range(N_HID):
                h_nxt = [[hpool.tile([128, 512], F32R, tag=f"h{oc}_{bt}",
                                     name=f"h{oc}_{bt}_e{e}l{l}")
                          for bt in range(NT)] for oc in range(2)]
                base = l * 2 * N_UNITS
                for bt in range(NT):
                    for oc in range(2):
                        ps = pspool.tile([128, 512], F32, tag="ps")
                        nc.tensor.matmul(
                            ps, lhsT=(wh_t[:, base + oc * 128: base + oc * 128 + 128]),
                            rhs=(h_cur[0][bt]), start=True, stop=False)
                        nc.tensor.matmul(
                            ps, lhsT=(wh_t[:, base + N_UNITS + oc * 128: base + N_UNITS + oc * 128 + 128]),
                            rhs=(h_cur[1][bt]), start=False, stop=True)
                        relu_post(h_nxt[oc][bt], ps,
                                  bh_t[:, l * 2 + oc: l * 2 + oc + 1],
                                  oc == 0)
                h_cur = h_nxt

            # ---- final layer: out[o', b] += sum_kc V[e][kc].T @ h[kc] ----
            # Adds are produced one per 2 matmuls (~920 ns) and served on DVE
            # in ~660 ns, so DVE never backs up; the next ensemble's L0 relus
            # go to ACT and never queue behind these. For the last ensemble,
            # each finished output slice DMAs out immediately (73 real rows
            # only for the upper chunk), overlapping the drain.
            # Last ensemble runs oc-major so out_t[0] is final mid-phase and
            # its 1MB DMA overlaps the oc1 matmuls; out_t[1] moves only the
            # OC1=73 real rows. Both DMAs are contiguous in DRAM -- a strided
            # or small dst fragments onto a single DMA ring (~10us/MB).
            last = e == ENS - 1
            order = ([(0, bt) for bt in range(NT)] + [(1, bt) for bt in range(NT)]
                     ) if last else [(oc, bt) for bt in range(NT) for oc in range(2)]
            for oc, bt in order:
                    sl = slice(bt * 512, (bt + 1) * 512)
                    ps = pspool.tile([128, 512], F32, tag="ps")
                    nc.tensor.matmul(ps, lhsT=(v_t[:, oc * 128: oc * 128 + 128]),
                                     rhs=(h_cur[0][bt]), start=True, stop=False)
                    nc.tensor.matmul(ps, lhsT=(v_t[:, 256 + oc * 128: 256 + oc * 128 + 128]),
                                     rhs=(h_cur[1][bt]), start=False, stop=True)
                    if e == 0:
                        nc.vector.tensor_scalar(out=out_t[oc][:, sl], in0=ps,
                                                scalar1=bp_t[:, oc:oc + 1],
                                                scalar2=None, op0=add)
                    elif not last:
                        nc.vector.tensor_tensor(out=out_t[oc][:, sl],
                                                in0=out_t[oc][:, sl],
                                                in1=ps, op=add)
                    else:
                        nc.vector.tensor_tensor(out=outb_t[oc][:, sl],
                                                in0=out_t[oc][:, sl],
                                                in1=ps, op=add)
                        if bt == NT - 1:
                            # stripe the output DMA across both HWDGE queues
                            # (SP + Act): one dma_start per 8 rows, contiguous
                            # in DRAM, so stripes pipeline on parallel rings
                            rows = 128 if oc == 0 else OC1
                            engs = [nc.sync, nc.scalar]
                            for r0 in range(0, rows, 8):
                                r1 = min(r0 + 8, rows)
                                eng = engs[(r0 // 8) % len(engs)]
                                eng.dma_start(
                                    out=yt[oc * 128 + r0: oc * 128 + r1, :],
                                    in_=outb_t[oc][r0:r1, :])

    nc.compile()
    return nc


def prepare_inputs(x, W0, b0, Wh, bh, Wf, bf):
    """Host-side weight refactoring + per-core sharding. All exact fp32
    linear algebra (bias folds + the channel-mix mean folded into Wf)."""
    x = np.asarray(x, np.float32)
    W0 = np.asarray(W0, np.float32)
    b0 = np.asarray(b0, np.float32)
    Wh = np.asarray(Wh, np.float32)
    bh = np.asarray(bh, np.float32)
    Wf = np.asarray(Wf, np.float32)
    bf = np.asarray(bf, np.float32)

    # layer 0 with bias folded: lhsT rows = 6 inputs + ones row
    w0a = np.concatenate([W0, b0[:, None, :]], axis=1)  # (ENS, 7, 256)
    w0a = np.ascontiguousarray(w0a)

    # hidden weights -> [e, p, (l, kc, o)]
    whh = (Wh.transpose(1, 0, 2, 3)              # (e, l, h, o)
             .reshape(ENS, N_HID, 2, 128, N_UNITS)
             .transpose(0, 3, 1, 2, 4)           # (e, p, l, kc, o)
             .reshape(ENS, 128, N_HID * 2 * N_UNITS))
    whh = np.ascontiguousarray(whh)

    # hidden biases -> [e, p, (l, oc)]
    bhh = (bh.transpose(1, 0, 2)                 # (e, l, o)
             .reshape(ENS, N_HID, 2, 128)
             .transpose(0, 3, 1, 2)              # (e, p, l, oc)
             .reshape(ENS, 128, N_HID * 2))
    bhh = np.ascontiguousarray(bhh)

    # fold the strided channel-mix mean into the final weights:
    # out[b, o'] = 0.1 * sum_j yflat[b, o'*10+j],  yflat col c = e*201+o
    C = ENS * OUT_DIM
    M = np.zeros((C, OUT_DIM), np.float32)
    M[np.arange(C), np.arange(C) // ENS] = 1.0 / ENS
    Me = M.reshape(ENS, OUT_DIM, OUT_DIM)
    V = np.einsum('eho,eoc->ehc', Wf, Me)        # (ENS, 256, 201)
    bpv = bf.reshape(C) @ M                      # (201,)

    Vp = np.zeros((ENS, N_UNITS, 256), np.float32)
    Vp[:, :, :OUT_DIM] = V
    vww = (Vp.reshape(ENS, 2, 128, 256)
             .transpose(0, 2, 1, 3)              # (e, p, kc, o')
             .reshape(ENS, 128, 2 * 256))
    vww = np.ascontiguousarray(vww)

    bp_pad = np.zeros(256, np.float32)
    bp_pad[:OUT_DIM] = bpv
    bp_t = np.ascontiguousarray(bp_pad.reshape(2, 128).T)  # (128, 2)

    ones = np.ones((1, BC), np.float32)
    in_maps = []
    for c in range(N_CORES):
        xs = x[c * BC:(c + 1) * BC]              # (BC, 6)
        xt = np.ascontiguousarray(
            np.concatenate([xs.T, ones], axis=0))  # (7, BC)
        in_maps.append({
            "xt": xt, "w0": w0a, "wh": whh, "bh": bhh, "vw": vww, "bp": bp_t,
        })
    return in_maps


def run(in_maps, trace=False, tmpdir=None):
    if "nc" not in _CACHE:
        _CACHE["nc"] = build_program()
    nc = _CACHE["nc"]
    res = bass_utils.run_bass_kernel_spmd(
        nc, in_maps, core_ids=list(range(N_CORES)), trace=trace, tmpdir=tmpdir)
    return res


def kernel(x, W0, b0, Wh, bh, Wf, bf):
    in_maps = prepare_inputs(x, W0, b0, Wh, bh, Wf, bf)
    res = run(in_maps)
    out = np.empty((BATCH, OUT_DIM), np.float32)
    for c in range(N_CORES):
        y = np.asarray(res.results[c]["yt"]).astype(np.float32)
        out[c * BC:(c + 1) * BC, :] = y[:OUT_DIM].T
    return out



# revision 39
# speedup vs baseline: 1.0692x; 1.0030x over previous
"""Ensemble-MLP (grouped 1x1 conv) Trainium2 kernel.

Computation (per batch row b):
  h = relu(x @ W0[e] + b0[e])             e = 0..9 ensembles, 256 units
  h = relu(h @ Wh[l,e] + bh[l,e])         l = 0..6 hidden layers
  y[e] = h @ Wf[e] + bf[e]                201 outputs per ensemble
  out[b, o'] = mean_j yflat[b, o'*10 + j] (strided channel mix, yflat = e*201+o)

Strategy:
  * Data parallel: batch 16384 -> 2048 rows per core on 8 cores. Weights
    replicated.
  * Activations live in SBUF transposed: H[channel, batch], channel on
    partitions (256 = 2 chunks of 128), batch on the free axis (2048).
  * Every layer is matmul(out[o, b] += W[c, o].T @ H[c, b]) with fp32r
    (full-rate fp32 PE mode, N=512 columns per PSUM bank).
  * Layer-0 bias is folded into the matmul via an all-ones row appended to
    x^T (K=7). Hidden biases ride the relu post-op (per-partition bias).
  * The final channel-mixing mean is folded into the last-layer weights on
    the host: out = sum_e H_e @ V[e] + bp, V[e] = Wf[e] @ Me[e] (exact
    linear algebra, no approximation).
  * relu post-ops alternate between ScalarE (ACT) and VectorE (DVE) so the
    PE stays the bottleneck; final-layer partials accumulate into SBUF.
  * Warmup matmuls on a zeroed tile ramp the PE p-state (0.65 -> 2.4 GHz)
    while the first weight DMAs land, so real matmuls start at full clock.
  * Ensemble-accumulation adds run on the otherwise-idle GpSimd engine
    (e0..e8), keeping ACT/DVE dedicated to relus: no post-op backlog at
    ensemble boundaries means no PE idle gap and no p-state reset.
  * Last ensemble accumulates on DVE/GpSimd and each output slice DMAs
    out as soon as it is final, shrinking the tail drain.
"""

import numpy as np
from contextlib import ExitStack

import concourse.bass as bass
import concourse.mybir as mybir
import concourse.tile as tile
from concourse import bacc, bass_utils

F32 = mybir.dt.float32
F32R = mybir.dt.float32r
BF16 = mybir.dt.bfloat16

ENS, N_UNITS, N_HID, IN_DIM, OUT_DIM, BATCH = 10, 256, 7, 6, 201, 16384
N_CORES = 8
BC = BATCH // N_CORES          # 2048 batch rows per core
NT = BC // 512                 # 4 moving-operand tiles of 512
OC1 = OUT_DIM - 128            # 73 rows in the second output chunk
WARMUP_MM = 8                  # PE p-state warmup matmuls during DMA head

_CACHE = {}


def build_program():
    nc = bacc.Bacc("TRN2", debug=False)

    xt = nc.dram_tensor("xt", (IN_DIM + 1, BC), F32R, kind="ExternalInput").ap()
    w0 = nc.dram_tensor("w0", (ENS, IN_DIM + 1, N_UNITS), F32R, kind="ExternalInput").ap()
    wh = nc.dram_tensor("wh", (ENS, 128, N_HID * 2 * N_UNITS), F32R, kind="ExternalInput").ap()
    bh = nc.dram_tensor("bh", (ENS, 128, N_HID * 2), F32, kind="ExternalInput").ap()
    vw = nc.dram_tensor("vw", (ENS, 128, 2 * 256), F32R, kind="ExternalInput").ap()
    bp = nc.dram_tensor("bp", (128, 2), F32, kind="ExternalInput").ap()
    # output leaves the chip in bf16 (halves the critical-path DMA; the
    # ~0.2% quantization of the final value is far inside the error budget)
    yt = nc.dram_tensor("yt", (256, BC), BF16, kind="ExternalOutput").ap()

    add = mybir.AluOpType.add
    mx = mybir.AluOpType.max
    relu = mybir.ActivationFunctionType.Relu

    with ExitStack() as ctx:
        tc = ctx.enter_context(tile.TileContext(nc))
        const = ctx.enter_context(tc.tile_pool(name="const", bufs=1))
        wpool = ctx.enter_context(tc.tile_pool(name="w", bufs=2))
        hpool = ctx.enter_context(tc.tile_pool(name="h", bufs=2))
        opool = ctx.enter_context(tc.tile_pool(name="acc", bufs=1))
        pspool = ctx.enter_context(tc.tile_pool(name="ps", bufs=8, space="PSUM"))

        x_t = const.tile([IN_DIM + 1, BC], F32R)
        bp_t = const.tile([128, 2], F32)
        out_t = [opool.tile([128, BC], F32, tag=f"out{i}", name=f"out{i}")
                 for i in range(2)]
        outb_t = [opool.tile([128, BC], BF16, tag=f"outb{i}", name=f"outb{i}")
                  for i in range(2)]

        def relu_post(dst, ps, bias_ap, use_act):
            # oc0 -> ACT, oc1 -> DVE for every layer (4+4 per layer, matching
            # the bt-major production/consumption order), final-layer adds all
            # on DVE. The killer stall this avoids: a relu queued on one
            # engine behind a burst of other work (in-order queues) idles the
            # PE, and any PE idle gap resets its p-state (~10 matmuls at 1.2
            # GHz instead of 2.4 to re-ramp).
            if use_act:
                nc.scalar.activation(out=dst, in_=ps, func=relu,
                                     bias=bias_ap if bias_ap is not None else 0.0)
            elif bias_ap is not None:
                nc.vector.tensor_scalar(out=dst, in0=ps, scalar1=bias_ap,
                                        scalar2=0.0, op0=add, op1=mx)
            else:
                nc.vector.tensor_scalar(out=dst, in0=ps, scalar1=0.0,
                                        scalar2=None, op0=mx)

        for e in range(ENS):
            w0_t = wpool.tile([IN_DIM + 1, N_UNITS], F32R, tag="w0")
            nc.sync.dma_start(out=w0_t, in_=w0[e])
            if e == 0:
                # x lands on a single DMA engine (only 7 partition rows), so
                # issue it right after w0 in b-tile slices: layer-0's first
                # matmuls only wait on their own slice.
                for bt in range(NT):
                    nc.sync.dma_start(out=x_t[:, bt * 512:(bt + 1) * 512],
                                      in_=xt[:, bt * 512:(bt + 1) * 512])
                nc.sync.dma_start(out=bp_t, in_=bp)
                # PE p-state ramps with *continuous* busy time (0.65 -> 1.2
                # -> 2.4 GHz over ~3us of execution). Dummy matmuls on the
                # just-landed w0 tile keep the PE busy through the remaining
                # DMA/queue-bring-up window so real matmuls start at full
                # clock (a cold PE costs ~5us over the first ~25 matmuls).
                for _ in range(WARMUP_MM):
                    ps = pspool.tile([128, 512], F32, tag="ps")
                    nc.tensor.matmul(ps[:, 0:N_UNITS], lhsT=w0_t[:, 0:128],
                                     rhs=w0_t, start=True, stop=True)
            # Each dma_start lands on ~one DMA ring at ~26 GB/s (2KB lines),
            # different dma_starts run on rings in parallel. Issue in the
            # order compute consumes (l0, H0 bias, l1, ...); for e0 only,
            # split l0/l1 into column chunks across several rings so H0/H1
            # don't wait out a full 256KB single-ring transfer.
            # wh/v use bufs=1: subtile WAR deps gate each layer-slice DMA on
            # THIS ensemble's own H_l reads, so e+1's weights stream in
            # phase-locked just-in-time (~26us lead, ~10us transfer) instead
            # of bursting 2MB at e+1's L0/H0 and slowing the PE's SBUF reads.
            wh_t = wpool.tile([128, N_HID * 2 * N_UNITS], F32R, tag="wh", bufs=1)
            bh_t = wpool.tile([128, N_HID * 2], F32, tag="bh")
            v_t = wpool.tile([128, 2 * 256], F32R, tag="v")
            nsplit = {0: 4, 1: 2} if e == 0 else {}
            for l in range(N_HID):
                ns = nsplit.get(l, 1)
                for hf in range(ns):
                    w = 512 // ns
                    s0 = l * 512 + hf * w
                    nc.sync.dma_start(out=wh_t[:, s0:s0 + w],
                                      in_=wh[e][:, s0:s0 + w])
                if l == 0:
                    nc.sync.dma_start(out=bh_t, in_=bh[e])
            # vw in 4 chunks on parallel rings: its transfer window otherwise
            # spans the next ensemble's L0/H0 and slows the PE's SBUF reads
            for vf in range(4):
                nc.sync.dma_start(out=v_t[:, vf * 128:(vf + 1) * 128],
                                  in_=vw[e][:, vf * 128:(vf + 1) * 128])

            # ---- layer 0: x^T (7, BC) -> h (2x128, BC); bias folded in ----
            # h is split per b-tile so cross-layer deps are slice-granular
            # (whole-tile deps stall the first matmuls of every layer).
            h_cur = [[hpool.tile([128, 512], F32R, tag=f"h{oc}_{bt}",
                                 name=f"h{oc}_{bt}_e{e}") for bt in range(NT)]
                     for oc in range(2)]
            # bt-major order everywhere: relus complete in exactly the order
            # the next layer's (bt-major) matmuls consume them
            for bt in range(NT):
                for oc in range(2):
                    sl = slice(bt * 512, (bt + 1) * 512)
                    ps = pspool.tile([128, 512], F32, tag="ps")
                    nc.tensor.matmul(ps, lhsT=(w0_t[:, oc * 128:(oc + 1) * 128]),
                                     rhs=(x_t[:, sl]), start=True, stop=True)
                    relu_post(h_cur[oc][bt], ps, None, oc == 0)

            # ---- 7 hidden layers: K=256 (2 chunks), M=256 (2 chunks) ----
            for l in range(N_HID):
                h_nxt = [[hpool.tile([128, 512], F32R, tag=f"h{oc}_{bt}",
                                     name=f"h{oc}_{bt}_e{e}l{l}")
                          for bt in range(NT)] for oc in range(2)]
                base = l * 2 * N_UNITS
                for bt in range(NT):
                    for oc in range(2):
                        ps = pspool.tile([128, 512], F32, tag="ps")
                        nc.tensor.matmul(
                            ps, lhsT=(wh_t[:, base + oc * 128: base + oc * 128 + 128]),
                            rhs=(h_cur[0][bt]), start=True, stop=False)
                        nc.tensor.matmul(
                            ps, lhsT=(wh_t[:, base + N_UNITS + oc * 128: base + N_UNITS + oc * 128 + 128]),
                            rhs=(h_cur[1][bt]), start=False, stop=True)
                        relu_post(h_nxt[oc][bt], ps,
                                  bh_t[:, l * 2 + oc: l * 2 + oc + 1],
                                  oc == 0)
                h_cur = h_nxt

            # ---- final layer: out[o', b] += sum_kc V[e][kc].T @ h[kc] ----
            # Adds are produced one per 2 matmuls (~920 ns) and served on DVE
            # in ~660 ns, so DVE never backs up; the next ensemble's L0 relus
            # go to ACT and never queue behind these. For the last ensemble,
            # each finished output slice DMAs out immediately (73 real rows
            # only for the upper chunk), overlapping the drain.
            # Last ensemble runs oc-major so out_t[0] is final mid-phase and
            # its 1MB DMA overlaps the oc1 matmuls; out_t[1] moves only the
            # OC1=73 real rows. Both DMAs are contiguous in DRAM -- a strided
            # or small dst fragments onto a single DMA ring (~10us/MB).
            last = e == ENS - 1
            order = ([(0, bt) for bt in range(NT)] + [(1, bt) for bt in range(NT)]
                     ) if last else [(oc, bt) for bt in range(NT) for oc in range(2)]
            for oc, bt in order:
                    sl = slice(bt * 512, (bt + 1) * 512)
                    ps = pspool.tile([128, 512], F32, tag="ps")
                    nc.tensor.matmul(ps, lhsT=(v_t[:, oc * 128: oc * 128 + 128]),
                                     rhs=(h_cur[0][bt]), start=True, stop=False)
                    nc.tensor.matmul(ps, lhsT=(v_t[:, 256 + oc * 128: 256 + oc * 128 + 128]),
                                     rhs=(h_cur[1][bt]), start=False, stop=True)
                    if e == 0:
                        nc.vector.tensor_scalar(out=out_t[oc][:, sl], in0=ps,
                                                scalar1=bp_t[:, oc:oc + 1],
                                                scalar2=None, op0=add)
                    elif not last:
                        nc.vector.tensor_tensor(out=out_t[oc][:, sl],
                                                in0=out_t[oc][:, sl],
                                                in1=ps, op=add)
                    else:
                        nc.vector.tensor_tensor(out=outb_t[oc][:, sl],
                                                in0=out_t[oc][:, sl],
                                                in1=ps, op=add)
                        if bt == NT - 1:
                            # stripe the output DMA across both HWDGE queues
                            # (SP + Act): one dma_start per 8 rows, contiguous
                            # in DRAM, so stripes pipeline on parallel rings
                            rows = 128 if oc == 0 else OC1
                            engs = [nc.sync, nc.scalar]
                            for r0 in range(0, rows, 8):
                                r1 = min(r0 + 8, rows)
                                eng = engs[(r0 // 8) % len(engs)]
                                eng.dma_start(
                                    out=yt[oc * 128 + r0: oc * 128 + r1, :],
                                    in_=outb_t[oc][r0:r1, :])

    nc.compile()
    return nc


def prepare_inputs(x, W0, b0, Wh, bh, Wf, bf):
    """Host-side weight refactoring + per-core sharding. All exact fp32
    linear algebra (bias folds + the channel-mix mean folded into Wf)."""
    x = np.asarray(x, np.float32)
    W0 = np.asarray(W0, np.float32)
    b0 = np.asarray(b0, np.float32)
    Wh = np.asarray(Wh, np.float32)
    bh = np.asarray(bh, np.float32)
    Wf = np.asarray(Wf, np.float32)
    bf = np.asarray(bf, np.float32)

    # layer 0 with bias folded: lhsT rows = 6 inputs + ones row
    w0a = np.concatenate([W0, b0[:, None, :]], axis=1)  # (ENS, 7, 256)
    w0a = np.ascontiguousarray(w0a)

    # hidden weights -> [e, p, (l, kc, o)]
    whh = (Wh.transpose(1, 0, 2, 3)              # (e, l, h, o)
             .reshape(ENS, N_HID, 2, 128, N_UNITS)
             .transpose(0, 3, 1, 2, 4)           # (e, p, l, kc, o)
             .reshape(ENS, 128, N_HID * 2 * N_UNITS))
    whh = np.ascontiguousarray(whh)

    # hidden biases -> [e, p, (l, oc)]
    bhh = (bh.transpose(1, 0, 2)                 # (e, l, o)
             .reshape(ENS, N_HID, 2, 128)
             .transpose(0, 3, 1, 2)              # (e, p, l, oc)
             .reshape(ENS, 128, N_HID * 2))
    bhh = np.ascontiguousarray(bhh)

    # fold the strided channel-mix mean into the final weights:
    # out[b, o'] = 0.1 * sum_j yflat[b, o'*10+j],  yflat col c = e*201+o
    C = ENS * OUT_DIM
    M = np.zeros((C, OUT_DIM), np.float32)
    M[np.arange(C), np.arange(C) // ENS] = 1.0 / ENS
    Me = M.reshape(ENS, OUT_DIM, OUT_DIM)
    V = np.einsum('eho,eoc->ehc', Wf, Me)        # (ENS, 256, 201)
    bpv = bf.reshape(C) @ M                      # (201,)

    Vp = np.zeros((ENS, N_UNITS, 256), np.float32)
    Vp[:, :, :OUT_DIM] = V
    vww = (Vp.reshape(ENS, 2, 128, 256)
             .transpose(0, 2, 1, 3)              # (e, p, kc, o')
             .reshape(ENS, 128, 2 * 256))
    vww = np.ascontiguousarray(vww)

    bp_pad = np.zeros(256, np.float32)
    bp_pad[:OUT_DIM] = bpv
    bp_t = np.ascontiguousarray(bp_pad.reshape(2, 128).T)  # (128, 2)

    ones = np.ones((1, BC), np.float32)
    in_maps = []
    for c in range(N_CORES):
        xs = x[c * BC:(c + 1) * BC]              # (BC, 6)
        xt = np.ascontiguousarray(
            np.concatenate([xs.T, ones], axis=0))  # (7, BC)
        in_maps.append({
            "xt": xt, "w0": w0a, "wh": whh, "bh": bhh, "vw": vww, "bp": bp_t,
        })
    return in_maps


def run(in_maps, trace=False, tmpdir=None):
    if "nc" not in _CACHE:
        _CACHE["nc"] = build_program()
    nc = _CACHE["nc"]
    res = bass_utils.run_bass_kernel_spmd(
        nc, in_maps, core_ids=list(range(N_CORES)), trace=trace, tmpdir=tmpdir)
    return res


def kernel(x, W0, b0, Wh, bh, Wf, bf):
    in_maps = prepare_inputs(x, W0, b0, Wh, bh, Wf, bf)
    res = run(in_maps)
    out = np.empty((BATCH, OUT_DIM), np.float32)
    for c in range(N_CORES):
        y = np.asarray(res.results[c]["yt"]).astype(np.float32)
        out[c * BC:(c + 1) * BC, :] = y[:OUT_DIM].T
    return out



# revision 41
# speedup vs baseline: 1.0801x; 1.0102x over previous
"""Ensemble-MLP (grouped 1x1 conv) Trainium2 kernel.

Computation (per batch row b):
  h = relu(x @ W0[e] + b0[e])             e = 0..9 ensembles, 256 units
  h = relu(h @ Wh[l,e] + bh[l,e])         l = 0..6 hidden layers
  y[e] = h @ Wf[e] + bf[e]                201 outputs per ensemble
  out[b, o'] = mean_j yflat[b, o'*10 + j] (strided channel mix, yflat = e*201+o)

Strategy:
  * Data parallel: batch 16384 -> 2048 rows per core on 8 cores. Weights
    replicated.
  * Activations live in SBUF transposed: H[channel, batch], channel on
    partitions (256 = 2 chunks of 128), batch on the free axis (2048).
  * Every layer is matmul(out[o, b] += W[c, o].T @ H[c, b]) with fp32r
    (full-rate fp32 PE mode, N=512 columns per PSUM bank).
  * Layer-0 bias is folded into the matmul via an all-ones row appended to
    x^T (K=7). Hidden biases ride the relu post-op (per-partition bias).
  * The final channel-mixing mean is folded into the last-layer weights on
    the host: out = sum_e H_e @ V[e] + bp, V[e] = Wf[e] @ Me[e] (exact
    linear algebra, no approximation).
  * relu post-ops alternate between ScalarE (ACT) and VectorE (DVE) so the
    PE stays the bottleneck; final-layer partials accumulate into SBUF.
  * Warmup matmuls on a zeroed tile ramp the PE p-state (0.65 -> 2.4 GHz)
    while the first weight DMAs land, so real matmuls start at full clock.
  * Ensemble-accumulation adds run on the otherwise-idle GpSimd engine
    (e0..e8), keeping ACT/DVE dedicated to relus: no post-op backlog at
    ensemble boundaries means no PE idle gap and no p-state reset.
  * Last ensemble accumulates on DVE/GpSimd and each output slice DMAs
    out as soon as it is final, shrinking the tail drain.
"""

import numpy as np
from contextlib import ExitStack

import concourse.bass as bass
import concourse.mybir as mybir
import concourse.tile as tile
from concourse import bacc, bass_utils

F32 = mybir.dt.float32
F32R = mybir.dt.float32r
BF16 = mybir.dt.bfloat16

ENS, N_UNITS, N_HID, IN_DIM, OUT_DIM, BATCH = 10, 256, 7, 6, 201, 16384
N_CORES = 8
BC = BATCH // N_CORES          # 2048 batch rows per core
NT = BC // 512                 # 4 moving-operand tiles of 512
OC1 = OUT_DIM - 128            # 73 rows in the second output chunk
WARMUP_MM = 14                 # PE p-state warmup matmuls during DMA head

_CACHE = {}


def build_program():
    nc = bacc.Bacc("TRN2", debug=False)

    xt = nc.dram_tensor("xt", (IN_DIM + 1, BC), F32R, kind="ExternalInput").ap()
    w0 = nc.dram_tensor("w0", (ENS, IN_DIM + 1, N_UNITS), F32R, kind="ExternalInput").ap()
    wh = nc.dram_tensor("wh", (ENS, 128, N_HID * 2 * N_UNITS), F32R, kind="ExternalInput").ap()
    bh = nc.dram_tensor("bh", (ENS, 128, N_HID * 2), F32, kind="ExternalInput").ap()
    vw = nc.dram_tensor("vw", (ENS, 128, 2 * 256), F32R, kind="ExternalInput").ap()
    bp = nc.dram_tensor("bp", (128, 2), F32, kind="ExternalInput").ap()
    # output leaves the chip in bf16 (halves the critical-path DMA; the
    # ~0.2% quantization of the final value is far inside the error budget)
    yt = nc.dram_tensor("yt", (256, BC), BF16, kind="ExternalOutput").ap()

    add = mybir.AluOpType.add
    mx = mybir.AluOpType.max
    relu = mybir.ActivationFunctionType.Relu

    with ExitStack() as ctx:
        tc = ctx.enter_context(tile.TileContext(nc))
        const = ctx.enter_context(tc.tile_pool(name="const", bufs=1))
        wpool = ctx.enter_context(tc.tile_pool(name="w", bufs=2))
        hpool = ctx.enter_context(tc.tile_pool(name="h", bufs=2))
        opool = ctx.enter_context(tc.tile_pool(name="acc", bufs=1))
        pspool = ctx.enter_context(tc.tile_pool(name="ps", bufs=8, space="PSUM"))

        x_t = const.tile([IN_DIM + 1, BC], F32R)
        bp_t = const.tile([128, 2], F32)
        out_t = [opool.tile([128, BC], F32, tag=f"out{i}", name=f"out{i}")
                 for i in range(2)]
        outb_t = [opool.tile([128, BC], BF16, tag=f"outb{i}", name=f"outb{i}")
                  for i in range(2)]

        def relu_post(dst, ps, bias_ap, use_act):
            # oc0 -> ACT, oc1 -> DVE for every layer (4+4 per layer, matching
            # the bt-major production/consumption order), final-layer adds all
            # on DVE. The killer stall this avoids: a relu queued on one
            # engine behind a burst of other work (in-order queues) idles the
            # PE, and any PE idle gap resets its p-state (~10 matmuls at 1.2
            # GHz instead of 2.4 to re-ramp).
            if use_act:
                nc.scalar.activation(out=dst, in_=ps, func=relu,
                                     bias=bias_ap if bias_ap is not None else 0.0)
            elif bias_ap is not None:
                nc.vector.tensor_scalar(out=dst, in0=ps, scalar1=bias_ap,
                                        scalar2=0.0, op0=add, op1=mx)
            else:
                nc.vector.tensor_scalar(out=dst, in0=ps, scalar1=0.0,
                                        scalar2=None, op0=mx)

        for e in range(ENS):
            w0_t = wpool.tile([IN_DIM + 1, N_UNITS], F32R, tag="w0")
            nc.sync.dma_start(out=w0_t, in_=w0[e])
            if e == 0:
                # x lands on a single DMA engine (only 7 partition rows), so
                # issue it right after w0 in b-tile slices: layer-0's first
                # matmuls only wait on their own slice.
                for bt in range(NT):
                    nc.sync.dma_start(out=x_t[:, bt * 512:(bt + 1) * 512],
                                      in_=xt[:, bt * 512:(bt + 1) * 512])
                nc.sync.dma_start(out=bp_t, in_=bp)
                # PE p-state ramps with *continuous* busy time (0.65 -> 1.2
                # -> 2.4 GHz over ~3us of execution). Dummy matmuls on the
                # just-landed w0 tile keep the PE busy through the remaining
                # DMA/queue-bring-up window so real matmuls start at full
                # clock (a cold PE costs ~5us over the first ~25 matmuls).
                for _ in range(WARMUP_MM):
                    ps = pspool.tile([128, 512], F32, tag="ps")
                    nc.tensor.matmul(ps[:, 0:N_UNITS], lhsT=w0_t[:, 0:128],
                                     rhs=w0_t, start=True, stop=True)
            # Each dma_start lands on ~one DMA ring at ~26 GB/s (2KB lines),
            # different dma_starts run on rings in parallel. Issue in the
            # order compute consumes (l0, H0 bias, l1, ...); for e0 only,
            # split l0/l1 into column chunks across several rings so H0/H1
            # don't wait out a full 256KB single-ring transfer.
            # wh/v use bufs=1: subtile WAR deps gate each layer-slice DMA on
            # THIS ensemble's own H_l reads, so e+1's weights stream in
            # phase-locked just-in-time (~26us lead, ~10us transfer) instead
            # of bursting 2MB at e+1's L0/H0 and slowing the PE's SBUF reads.
            wh_t = wpool.tile([128, N_HID * 2 * N_UNITS], F32R, tag="wh", bufs=1)
            bh_t = wpool.tile([128, N_HID * 2], F32, tag="bh")
            v_t = wpool.tile([128, 2 * 256], F32R, tag="v")
            nsplit = {0: 4, 1: 2} if e == 0 else {}
            for l in range(N_HID):
                ns = nsplit.get(l, 1)
                for hf in range(ns):
                    w = 512 // ns
                    s0 = l * 512 + hf * w
                    nc.sync.dma_start(out=wh_t[:, s0:s0 + w],
                                      in_=wh[e][:, s0:s0 + w])
                if l == 0:
                    nc.sync.dma_start(out=bh_t, in_=bh[e])
            # vw in 4 chunks on parallel rings: its transfer window otherwise
            # spans the next ensemble's L0/H0 and slows the PE's SBUF reads
            for vf in range(4):
                nc.sync.dma_start(out=v_t[:, vf * 128:(vf + 1) * 128],
                                  in_=vw[e][:, vf * 128:(vf + 1) * 128])

            # ---- layer 0: x^T (7, BC) -> h (2x128, BC); bias folded in ----
            # h is split per b-tile so cross-layer deps are slice-granular
            # (whole-tile deps stall the first matmuls of every layer).
            h_cur = [[hpool.tile([128, 512], F32R, tag=f"h{oc}_{bt}",
                                 name=f"h{oc}_{bt}_e{e}") for bt in range(NT)]
                     for oc in range(2)]
            # bt-major order everywhere: relus complete in exactly the order
            # the next layer's (bt-major) matmuls consume them
            for bt in range(NT):
                for oc in range(2):
                    sl = slice(bt * 512, (bt + 1) * 512)
                    ps = pspool.tile([128, 512], F32, tag="ps")
                    nc.tensor.matmul(ps, lhsT=(w0_t[:, oc * 128:(oc + 1) * 128]),
                                     rhs=(x_t[:, sl]), start=True, stop=True)
                    relu_post(h_cur[oc][bt], ps, None, oc == 0)

            # ---- 7 hidden layers: K=256 (2 chunks), M=256 (2 chunks) ----
            for l in range(N_HID):
                h_nxt = [[hpool.tile([128, 512], F32R, tag=f"h{oc}_{bt}",
                                     name=f"h{oc}_{bt}_e{e}l{l}")
                          for bt in range(NT)] for oc in range(2)]
                base = l * 2 * N_UNITS
                for bt in range(NT):
                    for oc in range(2):
                        ps = pspool.tile([128, 512], F32, tag="ps")
                        nc.tensor.matmul(
                            ps, lhsT=(wh_t[:, base + oc * 128: base + oc * 128 + 128]),
                            rhs=(h_cur[0][bt]), start=True, stop=False)
                        nc.tensor.matmul(
                            ps, lhsT=(wh_t[:, base + N_UNITS + oc * 128: base + N_UNITS + oc * 128 + 128]),
                            rhs=(h_cur[1][bt]), start=False, stop=True)
                        relu_post(h_nxt[oc][bt], ps,
                                  bh_t[:, l * 2 + oc: l * 2 + oc + 1],
                                  oc == 0)
                h_cur = h_nxt

            # ---- final layer: out[o', b] += sum_kc V[e][kc].T @ h[kc] ----
            # Adds are produced one per 2 matmuls (~920 ns) and served on DVE
            # in ~660 ns, so DVE never backs up; the next ensemble's L0 relus
            # go to ACT and never queue behind these. For the last ensemble,
            # each finished output slice DMAs out immediately (73 real rows
            # only for the upper chunk), overlapping the drain.
            # Last ensemble runs oc-major so out_t[0] is final mid-phase and
            # its 1MB DMA overlaps the oc1 matmuls; out_t[1] moves only the
            # OC1=73 real rows. Both DMAs are contiguous in DRAM -- a strided
            # or small dst fragments onto a single DMA ring (~10us/MB).
            last = e == ENS - 1
            order = ([(0, bt) for bt in range(NT)] + [(1, bt) for bt in range(NT)]
                     ) if last else [(oc, bt) for bt in range(NT) for oc in range(2)]
            for oc, bt in order:
                    sl = slice(bt * 512, (bt + 1) * 512)
                    ps = pspool.tile([128, 512], F32, tag="ps")
                    nc.tensor.matmul(ps, lhsT=(v_t[:, oc * 128: oc * 128 + 128]),
                                     rhs=(h_cur[0][bt]), start=True, stop=False)
                    nc.tensor.matmul(ps, lhsT=(v_t[:, 256 + oc * 128: 256 + oc * 128 + 128]),
                                     rhs=(h_cur[1][bt]), start=False, stop=True)
                    if e == 0:
                        nc.vector.tensor_scalar(out=out_t[oc][:, sl], in0=ps,
                                                scalar1=bp_t[:, oc:oc + 1],
                                                scalar2=None, op0=add)
                    elif not last:
                        nc.vector.tensor_tensor(out=out_t[oc][:, sl],
                                                in0=out_t[oc][:, sl],
                                                in1=ps, op=add)
                    else:
                        nc.vector.tensor_tensor(out=outb_t[oc][:, sl],
                                                in0=out_t[oc][:, sl],
                                                in1=ps, op=add)
                        if bt == NT - 1:
                            # stripe the output DMA across both HWDGE queues
                            # (SP + Act): one dma_start per 8 rows, contiguous
                            # in DRAM, so stripes pipeline on parallel rings
                            # 16-row stripes: few enough that per-dma_start
                            # queue dispatch (~0.8us each) doesn't dominate,
                            # small enough to pipeline rings on both queues
                            rows = 128 if oc == 0 else OC1
                            engs = [nc.sync, nc.scalar]
                            for r0 in range(0, rows, 16):
                                r1 = min(r0 + 16, rows)
                                eng = engs[(r0 // 16) % len(engs)]
                                eng.dma_start(
                                    out=yt[oc * 128 + r0: oc * 128 + r1, :],
                                    in_=outb_t[oc][r0:r1, :])

    nc.compile()
    return nc


def prepare_inputs(x, W0, b0, Wh, bh, Wf, bf):
    """Host-side weight refactoring + per-core sharding. All exact fp32
    linear algebra (bias folds + the channel-mix mean folded into Wf)."""
    x = np.asarray(x, np.float32)
    W0 = np.asarray(W0, np.float32)
    b0 = np.asarray(b0, np.float32)
    Wh = np.asarray(Wh, np.float32)
    bh = np.asarray(bh, np.float32)
    Wf = np.asarray(Wf, np.float32)
    bf = np.asarray(bf, np.float32)

    # layer 0 with bias folded: lhsT rows = 6 inputs + ones row
    w0a = np.concatenate([W0, b0[:, None, :]], axis=1)  # (ENS, 7, 256)
    w0a = np.ascontiguousarray(w0a)

    # hidden weights -> [e, p, (l, kc, o)]
    whh = (Wh.transpose(1, 0, 2, 3)              # (e, l, h, o)
             .reshape(ENS, N_HID, 2, 128, N_UNITS)
             .transpose(0, 3, 1, 2, 4)           # (e, p, l, kc, o)
             .reshape(ENS, 128, N_HID * 2 * N_UNITS))
    whh = np.ascontiguousarray(whh)

    # hidden biases -> [e, p, (l, oc)]
    bhh = (bh.transpose(1, 0, 2)                 # (e, l, o)
             .reshape(ENS, N_HID, 2, 128)
             .transpose(0, 3, 1, 2)              # (e, p, l, oc)
             .reshape(ENS, 128, N_HID * 2))
    bhh = np.ascontiguousarray(bhh)

    # fold the strided channel-mix mean into the final weights:
    # out[b, o'] = 0.1 * sum_j yflat[b, o'*10+j],  yflat col c = e*201+o
    C = ENS * OUT_DIM
    M = np.zeros((C, OUT_DIM), np.float32)
    M[np.arange(C), np.arange(C) // ENS] = 1.0 / ENS
    Me = M.reshape(ENS, OUT_DIM, OUT_DIM)
    V = np.einsum('eho,eoc->ehc', Wf, Me)        # (ENS, 256, 201)
    bpv = bf.reshape(C) @ M                      # (201,)

    Vp = np.zeros((ENS, N_UNITS, 256), np.float32)
    Vp[:, :, :OUT_DIM] = V
    vww = (Vp.reshape(ENS, 2, 128, 256)
             .transpose(0, 2, 1, 3)              # (e, p, kc, o')
             .reshape(ENS, 128, 2 * 256))
    vww = np.ascontiguousarray(vww)

    bp_pad = np.zeros(256, np.float32)
    bp_pad[:OUT_DIM] = bpv
    bp_t = np.ascontiguousarray(bp_pad.reshape(2, 128).T)  # (128, 2)

    ones = np.ones((1, BC), np.float32)
    in_maps = []
    for c in range(N_CORES):
        xs = x[c * BC:(c + 1) * BC]              # (BC, 6)
        xt = np.ascontiguousarray(
            np.concatenate([xs.T, ones], axis=0))  # (7, BC)
        in_maps.append({
            "xt": xt, "w0": w0a, "wh": whh, "bh": bhh, "vw": vww, "bp": bp_t,
        })
    return in_maps


def run(in_maps, trace=False, tmpdir=None):
    if "nc" not in _CACHE:
        _CACHE["nc"] = build_program()
    nc = _CACHE["nc"]
    res = bass_utils.run_bass_kernel_spmd(
        nc, in_maps, core_ids=list(range(N_CORES)), trace=trace, tmpdir=tmpdir)
    return res


def kernel(x, W0, b0, Wh, bh, Wf, bf):
    in_maps = prepare_inputs(x, W0, b0, Wh, bh, Wf, bf)
    res = run(in_maps)
    out = np.empty((BATCH, OUT_DIM), np.float32)
    for c in range(N_CORES):
        y = np.asarray(res.results[c]["yt"]).astype(np.float32)
        out[c * BC:(c + 1) * BC, :] = y[:OUT_DIM].T
    return out

